# revision 1
# baseline (speedup 1.0000x reference)
"""DeepSeek-MLA forward kernel for 8 Trainium2 NeuronCores (Bass/Tile).

Sharding: core c -> batch b = c // 4, head-group g = c % 4 (4 of 16 heads).
Each core computes its batch's down-projections (replicated x4 within the
batch group), its 4 heads' attention, and a partial output projection
y_part = out_heads_local @ w_o_local.  The host sums the 4 partials per
batch and stacks the 2 batches.

Precision: all matmul inputs are bf16 (fast PE weight-load path, halved x
DMA); all PSUM accumulation and the softmax/norm statistics are fp32.
Measured end-to-end relative error vs the fp32 reference: ~5.1e-3
(HW-verified; 344.7us on core 0).
"""

import os
import sys

import numpy as np

for _p in ("/opt/trn_rl_repo", "/root/.axon_site/_ro/trn_rl_repo"):
    if os.path.isdir(_p) and _p not in sys.path:
        sys.path.insert(0, _p)

import concourse.bass as bass
import concourse.mybir as mybir
import concourse.tile as tile
from concourse import bacc
B, S, D, H, DN, DR, R = 2, 2048, 2048, 16, 32, 32, 128
HD = DN + DR  # 64
EPS = 1e-5
NCORES = 8
NH = 4          # heads per core
SB = 512        # s-block (psum bank width in f32)
NSB = S // SB   # 4
ST = 128        # s-tile
NST = S // ST   # 16
KC = 128        # contraction chunk
NKC = D // KC   # 16
VW = HD + 1     # v columns incl. ones column (65)
F32 = mybir.dt.float32
F32R = mybir.dt.float32r
BF16 = mybir.dt.bfloat16


def _build_nc(causal: bool, use_mask: bool):
    nc = bacc.Bacc("TRN2", target_bir_lowering=False, debug=False,
                   num_devices=NCORES)

    xT = nc.dram_tensor("xT", [D, S], BF16, kind="ExternalInput").ap()
    wkv = nc.dram_tensor("wkv", [KC, D], BF16, kind="ExternalInput").ap()
    wq = nc.dram_tensor("wq", [KC, D], BF16, kind="ExternalInput").ap()
    kb = nc.dram_tensor("kb", [R, 2 * KC], BF16, kind="ExternalInput").ap()
    ksh = nc.dram_tensor("ksh", [R, 2 * KC], BF16, kind="ExternalInput").ap()
    qb = nc.dram_tensor("qb", [R, 2 * KC], BF16, kind="ExternalInput").ap()
    qsh = nc.dram_tensor("qsh", [R, 2 * KC], BF16, kind="ExternalInput").ap()
    uv = nc.dram_tensor("uv", [R, NH * HD], BF16, kind="ExternalInput").ap()
    wo = nc.dram_tensor("wo", [KC, 2 * D], BF16, kind="ExternalInput").ap()
    cosP = nc.dram_tensor("cosP", [128, S], F32, kind="ExternalInput").ap()
    sinP = nc.dram_tensor("sinP", [128, S], F32, kind="ExternalInput").ap()
    maskT = None
    if use_mask:
        maskT = nc.dram_tensor("maskT", [S, S], F32, kind="ExternalInput").ap()
    y = nc.dram_tensor("y", [S, D], F32, kind="ExternalOutput").ap()

    AF = mybir.ActivationFunctionType
    ALU = mybir.AluOpType

    with tile.TileContext(nc) as tc:
        from contextlib import ExitStack
        with ExitStack() as ctx:
            stat = ctx.enter_context(tc.tile_pool(name="static", bufs=1))
            # persistent SBUF tensors
            ckvT = stat.tile([R, S], BF16, name="ckvT")
            cqT = stat.tile([R, S], BF16, name="cqT")
            kT01 = stat.tile([128, S], BF16, name="kT01")
            kT23 = stat.tile([128, S], BF16, name="kT23")
            qT01 = stat.tile([128, S], BF16, name="qT01")
            qT23 = stat.tile([128, S], BF16, name="qT23")
            v_sb = stat.tile([128, NST * NH * VW], BF16, name="v_sb")
            outT01 = stat.tile([128, S], BF16, name="outT01")
            outT23 = stat.tile([128, S], BF16, name="outT23")
            kb_sb = stat.tile([R, 2 * KC], BF16, name="kb_sb")
            ksh_sb = stat.tile([R, 2 * KC], BF16, name="ksh_sb")
            qb_sb = stat.tile([R, 2 * KC], BF16, name="qb_sb")
            qsh_sb = stat.tile([R, 2 * KC], BF16, name="qsh_sb")
            uv_sb = stat.tile([R, NH * HD], BF16, name="uv_sb")
            wo_sb = stat.tile([KC, 2 * D], BF16, name="wo_sb")
            eps_sb = stat.tile([128, 1], F32, name="eps_sb")

            # static loads
            nc.sync.dma_start(kb_sb[:], kb)
            nc.sync.dma_start(ksh_sb[:], ksh)
            nc.sync.dma_start(qb_sb[:], qb)
            nc.sync.dma_start(qsh_sb[:], qsh)
            nc.sync.dma_start(uv_sb[:], uv)
            nc.sync.dma_start(wo_sb[:], wo)
            tri_sb = stat.tile([128, 128], BF16, name="tri_sb")
            onesf_sb = stat.tile([128, 64], F32, name="onesf_sb")
            nc.gpsimd.memset(onesf_sb[:], 1.0)
            nc.gpsimd.memset(eps_sb[:], EPS)
            # tri[p, f] = 1.0 if p <= f else 0.0 (keep-lower-triangle gate
            # for diagonal score strips in k-major layout)
            nc.gpsimd.memset(tri_sb[:], 1.0)
            nc.gpsimd.affine_select(
                out=tri_sb[:], in_=tri_sb[:], compare_op=ALU.is_ge,
                fill=0.0, base=0, channel_multiplier=-1, pattern=[[1, 128]])
            # ones column of v (col 64 of each 65-wide block)
            v_blocks = v_sb.rearrange("p (t h w) -> p t h w", t=NST, h=NH)
            nc.vector.tensor_copy(
                v_blocks[:, :, :, HD:VW],
                onesf_sb.rearrange("p (t h w) -> p t h w", t=NST, h=NH))

            # ---------------- Phase 1: c_kv^T, c_q^T + RMS norm ----------
            # Per-s-block chains end-to-end so downstream phases (which
            # depend on cT slices via subtile deps) pipeline behind P1.
            from concourse import bass_isa
            with tc.tile_pool(name="p1w", bufs=1) as p1w, \
                 tc.tile_pool(name="p1n", bufs=8) as p1n, \
                 tc.tile_pool(name="p1x", bufs=8) as p1x, \
                 tc.tile_pool(name="p1ps", bufs=6, space="PSUM") as p1ps:
                wkv_sb = p1w.tile([KC, D], BF16, name="wkv_sb")
                wq_sb = p1w.tile([KC, D], BF16, name="wq_sb")
                nc.sync.dma_start(wkv_sb[:], wkv)
                nc.sync.dma_start(wq_sb[:], wq)

                for sb in range(NSB):
                    sl = slice(sb * SB, (sb + 1) * SB)
                    cps = {nm: p1ps.tile([128, SB], F32, name=f"cps_{nm}",
                                         tag="cps") for nm in ("kv", "q")}
                    for k in range(NKC):
                        xt = p1x.tile([128, SB], BF16, name="xt", tag="xt")
                        nc.sync.dma_start(xt[:], xT[k * KC:(k + 1) * KC, sl])
                        for nm, wsb in (("kv", wkv_sb), ("q", wq_sb)):
                            nc.tensor.matmul(
                                cps[nm][:],
                                wsb[:, k * KC:(k + 1) * KC],
                                xt[:],
                                start=(k == 0), stop=(k == NKC - 1))
                    for nm, cT in (("kv", ckvT), ("q", cqT)):
                        sqt = p1n.tile([128, SB], F32, name="sqt", tag="sqt")
                        rst = p1n.tile([128, SB], F32, name="rst", tag="sqt")
                        nc.vector.tensor_copy(cT[:, sl], cps[nm][:])
                        nc.vector.tensor_mul(sqt[:], cT[:, sl], cT[:, sl])
                        nc.gpsimd.partition_all_reduce(
                            rst[:], sqt[:], channels=128,
                            reduce_op=bass_isa.ReduceOp.add)
                        # rstd = (sum/R + eps)^-1/2 = exp(-0.5 * ln(...)),
                        # both on ACT (vector.reciprocal is slow on HW)
                        nc.scalar.activation(rst[:], rst[:], AF.Ln,
                                             bias=eps_sb[:], scale=1.0 / R)
                        nc.scalar.activation(rst[:], rst[:], AF.Exp,
                                             scale=-0.5)
                        nc.vector.tensor_mul(cT[:, sl], cT[:, sl], rst[:])

            # ---------------- Phase 2: up-projections + rope -------------
            with tc.tile_pool(name="p2t", bufs=1) as p2t, \
                 tc.tile_pool(name="p2tmp", bufs=6) as p2tmp, \
                 tc.tile_pool(name="p2ps", bufs=6, space="PSUM") as p2ps, \
                 tc.tile_pool(name="p2vps", bufs=2, space="PSUM") as p2vps:
                cosP_sb = p2t.tile([128, S], F32, name="cosP_sb")
                sinP_sb = p2t.tile([128, S], F32, name="sinP_sb")
                nc.sync.dma_start(cosP_sb[:], cosP)
                nc.sync.dma_start(sinP_sb[:], sinP)

                # pair tensors: rows [he_nope|he_rope|ho_nope|ho_rope];
                # cosP rows are 1.0 (sinP rows 0.0) on nope rows so one fused
                # 3-op rope pass covers nope+rope together.
                for sb in range(NSB):
                    sl = slice(sb * SB, (sb + 1) * SB)
                    for cT, wb, wsh, dst in (
                            (ckvT, kb_sb, ksh_sb, (kT01, kT23)),
                            (cqT, qb_sb, qsh_sb, (qT01, qT23))):
                        for p in range(2):
                            pb = p2ps.tile([128, SB], F32, name="pb", tag="p2")
                            psh = p2ps.tile([128, SB], F32, name="psh", tag="p2")
                            nc.tensor.matmul(pb[:], wb[:, p * KC:(p + 1) * KC],
                                             cT[:, sl])
                            nc.tensor.matmul(psh[:], wsh[:, p * KC:(p + 1) * KC],
                                             cT[:, sl])
                            t1 = p2tmp.tile([128, SB], F32, name="t1", tag="t")
                            t2 = p2tmp.tile([128, SB], F32, name="t2", tag="t")
                            nc.vector.tensor_mul(t1[:], pb[:], cosP_sb[:, sl])
                            nc.vector.tensor_mul(t2[:], psh[:], sinP_sb[:, sl])
                            nc.vector.tensor_add(dst[p][:, sl], t1[:], t2[:])

                for t in range(NST):
                    vps = p2vps.tile([128, NH * HD], F32, name="vps", tag="v")
                    nc.tensor.matmul(vps[:], (ckvT[:, t * ST:(t + 1) * ST]),
                                     (uv_sb[:]))
                    dst = v_blocks[:, t, :, 0:HD]
                    src = vps.rearrange("p (h d) -> p h d", h=NH)
                    nc.vector.tensor_copy(dst, src)

            # ---------------- Phase 3: attention -------------------------
            with tc.tile_pool(name="p3e", bufs=8) as p3e, \
                 tc.tile_pool(name="p3m", bufs=3) as p3m, \
                 tc.tile_pool(name="p3rc", bufs=8) as p3rc, \
                 tc.tile_pool(name="p3rb", bufs=6) as p3rb, \
                 tc.tile_pool(name="p3sc", bufs=4, space="PSUM") as p3sc, \
                 tc.tile_pool(name="p3oa", bufs=4, space="PSUM") as p3oa:
                for j in range(NSB):
                    ktiles = list(range(4 * j + 4)) if causal else list(range(NST))
                    oacc = [p3oa.tile([VW, SB], F32, name=f"oa{j}_{h}", tag="oa")
                            for h in range(NH)]
                    for i in ktiles:
                        q0 = 128 * (i - 4 * j) if (causal and i >= 4 * j) else 0
                        qsl = slice(q0, SB)
                        mt = None
                        if use_mask:
                            mt = p3m.tile([128, SB], F32, name="mt", tag="mt")
                            nc.sync.dma_start(
                                mt[:], maskT[i * 128:(i + 1) * 128,
                                             j * SB:(j + 1) * SB])
                        for h in range(NH):
                            kTp = kT01 if h < 2 else kT23
                            qTp = qT01 if h < 2 else qT23
                            hs = slice((h % 2) * 64, (h % 2) * 64 + 64)
                            sc = p3sc.tile([128, SB], F32, name="sc", tag="sc")
                            nc.tensor.matmul(
                                sc[:, qsl],
                                kTp[hs, i * 128:(i + 1) * 128],
                                qTp[hs, j * SB + q0:(j + 1) * SB])
                            if use_mask:
                                nc.vector.tensor_add(sc[:, qsl], sc[:, qsl],
                                                     mt[:, qsl])
                            et = p3e.tile([128, SB], BF16, name="et", tag="e")
                            nc.scalar.activation(et[:, qsl], sc[:, qsl],
                                                 AF.Exp, scale=0.125)
                            if causal and i >= 4 * j:
                                nc.vector.tensor_mul(et[:, q0:q0 + 128],
                                                     et[:, q0:q0 + 128],
                                                     tri_sb[:])
                            nc.tensor.matmul(
                                oacc[h][:, qsl],
                                (v_sb[:, i * (NH * VW) + h * VW:
                                        i * (NH * VW) + (h + 1) * VW]),
                                (et[:, qsl]),
                                start=(i == ktiles[0]), stop=(i == ktiles[-1]))
                    for h in range(NH):
                        rc = p3rc.tile([1, SB], F32, name="rc", tag="rc")
                        nc.scalar.activation(rc[:], oacc[h][HD:VW, :], AF.Ln)
                        nc.scalar.activation(rc[:], rc[:], AF.Exp, scale=-1.0)
                        rb = p3rb.tile([HD, SB], F32, name="rb", tag="rb")
                        nc.gpsimd.partition_broadcast(rb[:], rc[:])
                        dst = (outT01 if h < 2 else outT23)[
                            (h % 2) * HD:(h % 2 + 1) * HD,
                            j * SB:(j + 1) * SB]
                        nc.vector.tensor_mul(dst, oacc[h][0:HD, :], rb[:])

            # ---------------- Phase 4: output projection ------------------
            with tc.tile_pool(name="p4y", bufs=4) as p4y, \
                 tc.tile_pool(name="p4ps", bufs=2, space="PSUM") as p4ps:
                for t in range(NST):
                    yp = p4ps.tile([128, S], F32, name="yp", tag="yp")
                    for db in range(NSB):
                        dsl = slice(db * SB, (db + 1) * SB)
                        for c, oT in ((0, outT01), (1, outT23)):
                            nc.tensor.matmul(
                                yp[:, dsl],
                                (oT[:, t * ST:(t + 1) * ST]),
                                (wo_sb[:, c * D + db * SB:c * D + (db + 1) * SB]),
                                start=(c == 0), stop=(c == 1))
                    ysb = p4y.tile([128, S], F32, name="ysb", tag="y")
                    nc.vector.tensor_copy(ysb[:], yp[:])
                    nc.sync.dma_start(y[t * ST:(t + 1) * ST, :], ysb[:])

    nc.finalize()
    return nc


_NC_CACHE = {}


def _get_nc(causal, use_mask):
    key = (causal, use_mask)
    if key not in _NC_CACHE:
        _NC_CACHE[key] = _build_nc(causal, use_mask)
    return _NC_CACHE[key]


def _prep_inputs(x, cos, sin, mask, w_kv_down, kv_norm_w, w_uk, w_ur, w_uv,
                 w_q_down, q_norm_w, w_uq, w_qr, w_o, use_mask):
    """Build the 8 per-core input maps (host-side shard + fold)."""
    f = np.float32
    x = np.asarray(x, f)
    cos = np.asarray(cos, f)
    sin = np.asarray(sin, f)
    w_kv_down = np.asarray(w_kv_down, f)
    w_q_down = np.asarray(w_q_down, f)
    kv_norm_w = np.asarray(kv_norm_w, f)
    q_norm_w = np.asarray(q_norm_w, f)
    w_uk_e = np.asarray(w_uk, f) * kv_norm_w[:, None]
    w_ur_e = np.asarray(w_ur, f) * kv_norm_w[:, None]
    w_uv_e = np.asarray(w_uv, f) * kv_norm_w[:, None]
    w_uq_e = np.asarray(w_uq, f) * q_norm_w[:, None]
    w_qr_e = np.asarray(w_qr, f) * q_norm_w[:, None]
    w_o = np.asarray(w_o, f)

    # shared rearrangements
    wkv = np.ascontiguousarray(
        w_kv_down.reshape(NKC, KC, R).transpose(1, 0, 2).reshape(KC, D))
    wq = np.ascontiguousarray(
        w_q_down.reshape(NKC, KC, R).transpose(1, 0, 2).reshape(KC, D))
    cosT = np.ascontiguousarray(cos.T)                 # [32, S]
    sinT = np.ascontiguousarray(sin.T)
    sinSg = np.concatenate([-sinT[:DR // 2], sinT[DR // 2:]], axis=0)
    one32 = np.ones((DR, S), np.float32)
    zero32 = np.zeros((DR, S), np.float32)
    # pair-tensor rope tables: nope rows pass through (cos=1, sin=0)
    cosPt = np.ascontiguousarray(
        np.concatenate([one32, cosT, one32, cosT], axis=0))
    sinPt = np.ascontiguousarray(
        np.concatenate([zero32, sinSg, zero32, sinSg], axis=0))
    # rope shift permutation within each head's 32 cols
    perm = np.concatenate([np.arange(16, 32), np.arange(0, 16)])

    import ml_dtypes as _md
    xTb = [np.ascontiguousarray(x[b].T).astype(_md.bfloat16) for b in range(B)]
    maskT8 = None
    if use_mask:
        m = np.asarray(mask, f).reshape(S, S)
        maskT8 = np.ascontiguousarray(m.T) * 8.0

    import ml_dtypes
    in_maps = []
    z32 = np.zeros((R, DN), np.float32)
    for core in range(NCORES):
        b, g = core // 4, core % 4
        cs = slice(g * NH * DN, (g + 1) * NH * DN)      # 128-wide col slice
        vs = slice(g * NH * HD, (g + 1) * NH * HD)      # 256-wide
        uk_l = w_uk_e[:, cs].reshape(R, NH, DN)
        ur_l = w_ur_e[:, cs].reshape(R, NH, DR)
        urs_l = ur_l[:, :, perm]
        uq_l = w_uq_e[:, cs].reshape(R, NH, DN)
        qr_l = w_qr_e[:, cs].reshape(R, NH, DR)
        qrs_l = qr_l[:, :, perm]
        # pair layout: [he_nope | he_rope | ho_nope | ho_rope] per 128 cols
        def pair(nope, rope):
            cols = []
            for h in range(NH):
                cols += [nope[:, h], rope[:, h]]
            return np.ascontiguousarray(np.concatenate(cols, axis=1))
        def pair_sh(sh):
            cols = []
            for h in range(NH):
                cols += [z32, sh[:, h]]
            return np.ascontiguousarray(np.concatenate(cols, axis=1))
        wo_loc = w_o[g * NH * HD:(g + 1) * NH * HD]     # [256, D]
        wo_r = np.ascontiguousarray(
            wo_loc.reshape(2, KC, D).transpose(1, 0, 2).reshape(KC, 2 * D)
        ).astype(ml_dtypes.bfloat16)
        m_ = {
            "xT": xTb[b],
            "wkv": wkv.astype(_md.bfloat16), "wq": wq.astype(_md.bfloat16),
            "kb": pair(uk_l, ur_l).astype(_md.bfloat16), "ksh": pair_sh(urs_l).astype(_md.bfloat16),
            "qb": pair(uq_l, qr_l).astype(_md.bfloat16), "qsh": pair_sh(qrs_l).astype(_md.bfloat16),
            "uv": np.ascontiguousarray(w_uv_e[:, vs]).astype(_md.bfloat16),
            "wo": wo_r,
            "cosP": cosPt, "sinP": sinPt,
        }
        if use_mask:
            m_["maskT"] = maskT8
        in_maps.append(m_)
    return in_maps


def _classify_mask(mask):
    m = np.asarray(mask, np.float32).reshape(S, S)
    if not np.any(m):
        return False, False          # dense, no mask
    causal_ref = np.where(
        np.tril(np.ones((S, S), dtype=bool)), np.float32(0.0),
        np.float32(-1e9))
    if np.array_equal(m, causal_ref):
        return True, False           # structural causal
    return False, True               # generic additive mask


LAST_RESULTS = None


def kernel(**inputs):
    global LAST_RESULTS
    from concourse.bass_utils import run_bass_kernel_spmd
    causal, use_mask = _classify_mask(inputs["mask"])
    nc = _get_nc(causal, use_mask)
    in_maps = _prep_inputs(
        inputs["x"], inputs["cos"], inputs["sin"], inputs["mask"],
        inputs["w_kv_down"], inputs["kv_norm_w"], inputs["w_uk"],
        inputs["w_ur"], inputs["w_uv"], inputs["w_q_down"],
        inputs["q_norm_w"], inputs["w_uq"], inputs["w_qr"], inputs["w_o"],
        use_mask)
    res = run_bass_kernel_spmd(nc, in_maps, list(range(NCORES)))
    LAST_RESULTS = res
    parts = [res.results[c]["y"] for c in range(NCORES)]
    out = np.empty((B, S, D), np.float32)
    for b in range(B):
        out[b] = parts[4 * b] + parts[4 * b + 1] + parts[4 * b + 2] + parts[4 * b + 3]
    return out



# revision 13
# speedup vs baseline: 1.3509x; 1.3509x over previous
"""DeepSeek-MLA forward kernel for 8 Trainium2 NeuronCores (Bass/Tile).

Sharding: core c -> batch b = c // 4, head-group g = c % 4 (4 of 16 heads).
Each core computes its batch's down-projections, its 4 heads' attention,
and a partial output projection; the host sums the 4 partials per batch.

Design notes (v2, restructured from the 344us baseline):
- All of x is streamed to SBUF up front in per-s-block contiguous chunks;
  phase 1 (down-proj) runs as one uninterrupted PE stream.
- KV-side RMS norm is deferred: c_kv stays unnormalized; 1/rms(c_kv) enters
  as the per-partition (per-key) scale vector of the softmax exp and as the
  per-partition scale of the V tiles (additionally folded with the 1/8
  softmax temperature).  Q-side norm is applied in phase 1 via a
  DMA-broadcast row multiply.
- Norm statistics use PE ones-matmuls (row + column orientations) instead
  of slow gpsimd partition reduce/broadcast ops.
- Attention processes head PAIRS in [128,1024] two-bank PSUM tiles: one
  exp ACTIVATE per pair halves the ACT instruction count.  The softmax
  reciprocal runs on DVE (reciprocal_approx_fast) so the ACT engine never
  switches activation tables mid-stream.
- The output projection (P4) of block j-1 is emitted interleaved into the
  attention waves of block j as PE filler work, keeping the tensor engine
  continuously busy (p-state ramp to 2.4 GHz).
- y partials are stored fp16 (halves DMA); host sums in fp32.
"""

import os
import sys

import numpy as np

for _p in ("/opt/trn_rl_repo", "/root/.axon_site/_ro/trn_rl_repo"):
    if os.path.isdir(_p) and _p not in sys.path:
        sys.path.insert(0, _p)

import concourse.bass as bass
import concourse.mybir as mybir
import concourse.tile as tile
from concourse import bacc

B, S, D, H, DN, DR, R = 2, 2048, 2048, 16, 32, 32, 128
HD = DN + DR  # 64
EPS = 1e-5
NCORES = 8
NH = 4          # heads per core
SB = 512        # s-block (psum bank width in f32)
NSB = S // SB   # 4
ST = 128        # s-tile
NST = S // ST   # 16
KC = 128        # contraction chunk
NKC = D // KC   # 16
VW = HD + 1     # v columns incl. eighth column (65)
F32 = mybir.dt.float32
BF16 = mybir.dt.bfloat16
F16 = mybir.dt.float16
LN_EIGHTH = float(np.log(0.125))


def _build_nc(causal: bool, use_mask: bool):
    nc = bacc.Bacc("TRN2", target_bir_lowering=False, debug=False,
                   num_devices=NCORES)

    # x stored s-block-major: [NSB, D, SB] so each block's 16 row-chunks are
    # contiguous 128KB DMAs and phase 1 can start after ~1 chunk.
    xT4 = nc.dram_tensor("xT4", [NSB, D, SB], BF16, kind="ExternalInput").ap()
    wkv = nc.dram_tensor("wkv", [KC, D], BF16, kind="ExternalInput").ap()
    wq = nc.dram_tensor("wq", [KC, D], BF16, kind="ExternalInput").ap()
    kb = nc.dram_tensor("kb", [R, 2 * KC], BF16, kind="ExternalInput").ap()
    ksh = nc.dram_tensor("ksh", [R, 2 * KC], BF16, kind="ExternalInput").ap()
    qb = nc.dram_tensor("qb", [R, 2 * KC], BF16, kind="ExternalInput").ap()
    qsh = nc.dram_tensor("qsh", [R, 2 * KC], BF16, kind="ExternalInput").ap()
    uv = nc.dram_tensor("uv", [R, NH * HD], BF16, kind="ExternalInput").ap()
    wo = nc.dram_tensor("wo", [KC, 2 * D], BF16, kind="ExternalInput").ap()
    # rope tables, s-block interleaved: per sb, cols [0:512]=cos, [512:1024]=sin
    cssin = nc.dram_tensor("cssin", [128, 2 * S], F32, kind="ExternalInput").ap()
    maskT = None
    if use_mask:
        maskT = nc.dram_tensor("maskT", [S, S], F32, kind="ExternalInput").ap()
    y = nc.dram_tensor("y", [S, D], F16, kind="ExternalOutput").ap()
    dbg = {}
    if os.environ.get("MLA_DEBUG"):
        for nm, shp, dt in (("d_ckvT", [R, S], BF16), ("d_cqT", [R, S], BF16),
                            ("d_kT01", [128, S], BF16), ("d_kT23", [128, S], BF16),
                            ("d_qT01", [128, S], BF16), ("d_qT23", [128, S], BF16),
                            ("d_rstdq", [1, S], F32), ("d_rstdk8", [128, NST], F32),
                            ("d_vsb", [128, NST * NH * VW], BF16),
                            ("d_outT01", [128, S], BF16), ("d_outT23", [128, S], BF16),
                            ("d_den", [1, S], F32), ("d_rc", [1, S], F32)):
            dbg[nm] = nc.dram_tensor(nm, shp, dt, kind="ExternalOutput").ap()

    AF = mybir.ActivationFunctionType
    ALU = mybir.AluOpType

    with tile.TileContext(nc) as tc:
        from contextlib import ExitStack
        with ExitStack() as ctx:
            stat = ctx.enter_context(tc.tile_pool(name="static", bufs=1))
            # persistent SBUF tensors
            xall = stat.tile([128, NKC * S], BF16, name="xall")
            ckvT = stat.tile([R, S], BF16, name="ckvT")
            cqT = stat.tile([R, S], BF16, name="cqT")
            kT01 = stat.tile([128, S], BF16, name="kT01")
            kT23 = stat.tile([128, S], BF16, name="kT23")
            qT01 = stat.tile([128, S], BF16, name="qT01")
            qT23 = stat.tile([128, S], BF16, name="qT23")
            v_sb = stat.tile([128, NST * NH * VW], BF16, name="v_sb")
            outT01 = stat.tile([128, S], BF16, name="outT01")
            outT23 = stat.tile([128, S], BF16, name="outT23")
            wkv_sb = stat.tile([KC, D], BF16, name="wkv_sb")
            wq_sb = stat.tile([KC, D], BF16, name="wq_sb")
            kb_sb = stat.tile([R, 2 * KC], BF16, name="kb_sb")
            ksh_sb = stat.tile([R, 2 * KC], BF16, name="ksh_sb")
            qb_sb = stat.tile([R, 2 * KC], BF16, name="qb_sb")
            qsh_sb = stat.tile([R, 2 * KC], BF16, name="qsh_sb")
            uv_sb = stat.tile([R, NH * HD], BF16, name="uv_sb")
            wo_sb = stat.tile([KC, 2 * D], BF16, name="wo_sb")
            cssin_sb = stat.tile([128, 2 * S], F32, name="cssin_sb")
            tri_sb = stat.tile([128, 128], BF16, name="tri_sb")
            ones_col = stat.tile([128, 1], BF16, name="ones_col")
            rstdk8 = stat.tile([128, NST], F32, name="rstdk8")
            rstdq = stat.tile([1, S], F32, name="rstdq")
            rstdkv = stat.tile([1, S], F32, name="rstdkv") if use_mask else None
            eps_sb = stat.tile([128, 1], F32, name="eps_sb")
            dendump = stat.tile([1, S], F32, name="dendump") if dbg else None
            rcdump = stat.tile([1, S], F32, name="rcdump") if dbg else None
            ln8_sb = stat.tile([128, 1], F32, name="ln8_sb")

            # ---- static loads (wkv/wq first: phase 1 needs them) ----
            nc.sync.dma_start(wkv_sb[:], wkv)
            nc.sync.dma_start(wq_sb[:], wq)
            # x, s-block-major so P1(0) is gated only on the first 16 chunks
            for sb in range(NSB):
                for k in range(NKC):
                    nc.sync.dma_start(
                        xall[:, (sb * NKC + k) * SB:(sb * NKC + k + 1) * SB],
                        xT4[sb, k * KC:(k + 1) * KC, :])
            nc.sync.dma_start(kb_sb[:], kb)
            nc.sync.dma_start(ksh_sb[:], ksh)
            nc.sync.dma_start(qb_sb[:], qb)
            nc.sync.dma_start(qsh_sb[:], qsh)
            nc.sync.dma_start(uv_sb[:], uv)
            nc.sync.dma_start(wo_sb[:], wo)
            nc.sync.dma_start(cssin_sb[:], cssin)

            nc.gpsimd.memset(ones_col[:], 1.0)
            nc.gpsimd.memset(eps_sb[:], EPS)
            nc.gpsimd.memset(ln8_sb[:], LN_EIGHTH)
            # v gets pre-filled with 0.125: the "denominator" column of each
            # head block stays 0.125 so PV accumulates denom/8 and the DVE
            # reciprocal directly yields 8/denom (the 8 un-does the 1/8
            # temperature folded into the V scale).
            nc.gpsimd.memset(v_sb[:], 0.125)
            nc.gpsimd.memset(tri_sb[:], 1.0)
            nc.gpsimd.affine_select(
                out=tri_sb[:], in_=tri_sb[:], compare_op=ALU.is_ge,
                fill=0.0, base=0, channel_multiplier=-1, pattern=[[1, 128]])

            v_blocks = v_sb.rearrange("p (t h w) -> p t h w", t=NST, h=NH)

            # pools
            pair = ctx.enter_context(
                tc.tile_pool(name="pair", bufs=2, space="PSUM"))
            oaccp = ctx.enter_context(
                tc.tile_pool(name="oaccp", bufs=4, space="PSUM"))
            sqp = ctx.enter_context(tc.tile_pool(name="sqp", bufs=2))
            t12p = ctx.enter_context(tc.tile_pool(name="t12p", bufs=2))
            etp = ctx.enter_context(tc.tile_pool(name="etp", bufs=4))
            rcp = ctx.enter_context(tc.tile_pool(name="rcp", bufs=3))
            tinyp = ctx.enter_context(tc.tile_pool(name="tinyp", bufs=2))
            rbqp = ctx.enter_context(tc.tile_pool(name="rbqp", bufs=2))
            rbop = ctx.enter_context(tc.tile_pool(name="rbop", bufs=3))
            ysbp = ctx.enter_context(tc.tile_pool(name="ysbp", bufs=3))
            mtp = ctx.enter_context(tc.tile_pool(name="mtp", bufs=4)) \
                if use_mask else None

            # ================= Phase 1: down-projections + norms ==========
            for sb in range(NSB):
                sl = slice(sb * SB, (sb + 1) * SB)
                cps = pair.tile([128, 2 * SB], F32, name="cps", tag="pair")
                for k in range(NKC):
                    xsl = xall[:, (sb * NKC + k) * SB:(sb * NKC + k + 1) * SB]
                    nc.tensor.matmul(cps[:, 0:SB],
                                     wkv_sb[:, k * KC:(k + 1) * KC], xsl,
                                     start=(k == 0), stop=(k == NKC - 1))
                    nc.tensor.matmul(cps[:, SB:2 * SB],
                                     wq_sb[:, k * KC:(k + 1) * KC], xsl,
                                     start=(k == 0), stop=(k == NKC - 1))
                sq = sqp.tile([128, 2 * SB], BF16, name="sq", tag="sq")
                nc.scalar.activation(sq[:], cps[:], AF.Square)
                small = pair.tile([128, 2 * SB], F32, name="small", tag="pair")
                # q-side row sums: [1, 512]
                nc.tensor.matmul(small[0:1, 0:SB], ones_col[:], sq[:, SB:2 * SB])
                if use_mask:
                    # kv-side row sums too (kv norm not deferred on mask path)
                    nc.tensor.matmul(small[1:2, 0:SB], ones_col[:], sq[:, 0:SB])
                else:
                    # kv-side column sums: [128, 1] per s-tile
                    for t in range(4):
                        nc.tensor.matmul(small[:, SB + t:SB + t + 1],
                                         sq[:, t * ST:(t + 1) * ST], ones_col[:])
                tq = rcp.tile([1, SB], F32, name="tq", tag="rc")
                nc.scalar.activation(tq[:], small[0:1, 0:SB], AF.Ln,
                                     bias=eps_sb[0:1, :], scale=1.0 / R)
                nc.scalar.activation(rstdq[0:1, sl], tq[:], AF.Exp, scale=-0.5)
                if use_mask:
                    tkv = rcp.tile([1, SB], F32, name="tkv", tag="rc")
                    nc.scalar.activation(tkv[:], small[1:2, 0:SB], AF.Ln,
                                         bias=eps_sb[0:1, :], scale=1.0 / R)
                    nc.scalar.activation(rstdkv[0:1, sl], tkv[:], AF.Exp,
                                         scale=-0.5)
                else:
                    tk = tinyp.tile([128, 4], F32, name="tk", tag="tiny")
                    nc.scalar.activation(tk[:], small[:, SB:SB + 4], AF.Ln,
                                         bias=eps_sb[:], scale=1.0 / R)
                    nc.scalar.activation(rstdk8[:, sb * 4:(sb + 1) * 4], tk[:],
                                         AF.Exp, scale=-0.5, bias=ln8_sb[:])
                rbq = rbqp.tile([128, SB], F32, name="rbq", tag="rbq")
                nc.gpsimd.partition_broadcast(rbq[:], rstdq[0:1, sl])
                nc.vector.tensor_mul(cqT[:, sl], cps[:, SB:2 * SB], rbq[:])
                if use_mask:
                    rbkv = rbqp.tile([128, SB], F32, name="rbkv", tag="rbq")
                    nc.gpsimd.partition_broadcast(rbkv[:], rstdkv[0:1, sl])
                    nc.vector.tensor_mul(ckvT[:, sl], cps[:, 0:SB], rbkv[:])
                else:
                    nc.vector.tensor_copy(ckvT[:, sl], cps[:, 0:SB])

            # ================= Fused P2 -> P3 -> P4 pipeline ==============
            def emit_p2(sb):
                sl = slice(sb * SB, (sb + 1) * SB)
                cs_sl = cssin_sb[:, sb * 2 * SB:(sb + 1) * 2 * SB]
                for cT, wb, wsh, dsts in (
                        (ckvT, kb_sb, ksh_sb, (kT01, kT23)),
                        (cqT, qb_sb, qsh_sb, (qT01, qT23))):
                    for p in range(2):
                        pp = pair.tile([128, 2 * SB], F32, name="pp",
                                       tag="pair")
                        nc.tensor.matmul(pp[:, 0:SB],
                                         wb[:, p * KC:(p + 1) * KC], cT[:, sl])
                        nc.tensor.matmul(pp[:, SB:2 * SB],
                                         wsh[:, p * KC:(p + 1) * KC], cT[:, sl])
                        t12 = t12p.tile([128, 2 * SB], F32, name="t12",
                                        tag="t12")
                        nc.vector.tensor_mul(t12[:], pp[:], cs_sl)
                        nc.vector.tensor_add(dsts[p][:, sl], t12[:, 0:SB],
                                             t12[:, SB:2 * SB])
                # v tiles for this block's 4 s-tiles
                vps = pair.tile([128, 2 * SB], F32, name="vps", tag="pair")
                for t in range(4):
                    g = sb * 4 + t
                    nc.tensor.matmul(vps[:, t * 256:(t + 1) * 256],
                                     ckvT[:, g * ST:(g + 1) * ST], uv_sb[:])
                for t in range(4):
                    g = sb * 4 + t
                    src = vps[:, t * 256:(t + 1) * 256].rearrange(
                        "p (h d) -> p h d", h=NH)
                    scal = 0.125 if use_mask else rstdk8[:, g:g + 1]
                    nc.vector.tensor_scalar_mul(
                        v_blocks[:, g, :, 0:HD], src, scal)

            def p4_units(j):
                units = []
                for t in range(4 * j, 4 * j + 4):
                    for dh in range(2):
                        def emit(t=t, dh=dh):
                            yp = pair.tile([128, 2 * SB], F32, name="yp",
                                           tag="pair")
                            for di in range(2):
                                dcol = dh * 1024 + di * SB
                                for c, oT in ((0, outT01), (1, outT23)):
                                    nc.tensor.matmul(
                                        yp[:, di * SB:(di + 1) * SB],
                                        oT[:, t * ST:(t + 1) * ST],
                                        wo_sb[:, c * D + dcol:c * D + dcol + SB],
                                        start=(c == 0), stop=(c == 1))
                            ysb = ysbp.tile([128, 2 * SB], F16, name="ysb",
                                            tag="ysb")
                            nc.vector.tensor_copy(ysb[:], yp[:])
                            nc.sync.dma_start(
                                y[t * ST:(t + 1) * ST,
                                  dh * 1024:(dh + 1) * 1024], ysb[:])
                        units.append(emit)
                return units

            def emit_scores(j, i):
                """Emit score matmuls + exp (+mask/tri) for key-tile i of
                block j, both head pairs. Returns the et tiles."""
                q0 = ST * (i - 4 * j) if (causal and i >= 4 * j) else 0
                ets = []
                mt = None
                if use_mask:
                    mt = mtp.tile([128, SB], F32, name="mt", tag="mt")
                    nc.sync.dma_start(
                        mt[:], maskT[i * ST:(i + 1) * ST,
                                     j * SB:(j + 1) * SB])
                for hp, (kTp, qTp) in enumerate(((kT01, qT01), (kT23, qT23))):
                    sc2 = pair.tile([128, 2 * SB], F32, name="sc2", tag="pair")
                    for half in range(2):
                        hs = slice(half * 64, half * 64 + 64)
                        nc.tensor.matmul(
                            sc2[:, half * SB + q0:(half + 1) * SB],
                            kTp[hs, i * ST:(i + 1) * ST],
                            qTp[hs, j * SB + q0:(j + 1) * SB])
                    if use_mask:
                        for half in range(2):
                            nc.vector.tensor_add(
                                sc2[:, half * SB:(half + 1) * SB],
                                sc2[:, half * SB:(half + 1) * SB], mt[:])
                    et = etp.tile([128, 2 * SB], BF16, name="et", tag="et")
                    scal = 0.125 if use_mask else rstdk8[:, i:i + 1]
                    if q0 == 0:
                        nc.scalar.activation(et[:], sc2[:], AF.Exp, scale=scal)
                    else:
                        for half in range(2):
                            rg = slice(half * SB + q0, (half + 1) * SB)
                            nc.scalar.activation(et[:, rg], sc2[:, rg],
                                                 AF.Exp, scale=scal)
                    if causal and i >= 4 * j:
                        for half in range(2):
                            rg = slice(half * SB + q0, half * SB + q0 + ST)
                            nc.vector.tensor_mul(et[:, rg], et[:, rg], tri_sb[:])
                    ets.append((et, q0))
                return ets

            def emit_pv(j, i, ets, first, last):
                for hp, (et, q0) in enumerate(ets):
                    for half in range(2):
                        h = 2 * hp + half
                        nc.tensor.matmul(
                            oacc[h][:, q0:SB],
                            v_sb[:, i * (NH * VW) + h * VW:
                                 i * (NH * VW) + (h + 1) * VW],
                            et[:, half * SB + q0:(half + 1) * SB],
                            start=first, stop=last)

            for j in range(NSB):
                emit_p2(j)
                ktiles = list(range(4 * j + 4)) if causal else list(range(NST))
                oacc = [oaccp.tile([VW, SB], F32, name=f"oa{h}", tag="oa")
                        for h in range(NH)]
                units = p4_units(j - 1) if j > 0 else []
                ui = 0
                prev = None
                for i in ktiles:
                    ets = emit_scores(j, i)
                    if ui < len(units):
                        units[ui]()
                        ui += 1
                    if prev is not None:
                        emit_pv(j, prev[0], prev[1],
                                prev[0] == ktiles[0], False)
                    prev = (i, ets)
                emit_pv(j, prev[0], prev[1], prev[0] == ktiles[0], True)
                while ui < len(units):
                    units[ui]()
                    ui += 1
                # block tail: softmax denominators + output scaling
                for h in range(NH):
                    den = rcp.tile([1, SB], F32, name="den", tag="rc")
                    nc.vector.tensor_copy(den[:], oacc[h][HD:VW, :])
                    rc = rcp.tile([1, SB], F32, name="rc", tag="rc")
                    nc.vector.reciprocal_approx_fast(out=rc[:], in_=den[:])
                    if dbg and h < 1:
                        o0 = j * SB
                        nc.vector.tensor_copy(
                            dendump[0:1, o0:o0 + SB], oacc[h][HD:VW, :])
                        nc.vector.tensor_copy(
                            rcdump[0:1, o0:o0 + SB], rc[:])
                    rbo = rbop.tile([HD, SB], F32, name="rbo", tag="rbo")
                    nc.gpsimd.partition_broadcast(rbo[:], rc[:])
                    dst = (outT01 if h < 2 else outT23)[
                        (h % 2) * HD:(h % 2 + 1) * HD,
                        j * SB:(j + 1) * SB]
                    nc.vector.tensor_mul(dst, oacc[h][0:HD, :], rbo[:])
            for u in p4_units(NSB - 1):
                u()
            if dbg:
                for nm, t in (("d_ckvT", ckvT), ("d_cqT", cqT),
                              ("d_kT01", kT01), ("d_kT23", kT23),
                              ("d_qT01", qT01), ("d_qT23", qT23),
                              ("d_rstdq", rstdq), ("d_rstdk8", rstdk8),
                              ("d_vsb", v_sb),
                              ("d_outT01", outT01), ("d_outT23", outT23),
                              ("d_den", dendump), ("d_rc", rcdump)):
                    nc.sync.dma_start(dbg[nm], t[:])

    nc.finalize()
    return nc


_NC_CACHE = {}


def _get_nc(causal, use_mask):
    key = (causal, use_mask)
    if key not in _NC_CACHE:
        _NC_CACHE[key] = _build_nc(causal, use_mask)
    return _NC_CACHE[key]


def _prep_inputs(x, cos, sin, mask, w_kv_down, kv_norm_w, w_uk, w_ur, w_uv,
                 w_q_down, q_norm_w, w_uq, w_qr, w_o, use_mask):
    """Build the 8 per-core input maps (host-side shard + fold)."""
    import ml_dtypes as md
    f = np.float32
    x = np.asarray(x, f)
    cos = np.asarray(cos, f)
    sin = np.asarray(sin, f)
    w_kv_down = np.asarray(w_kv_down, f)
    w_q_down = np.asarray(w_q_down, f)
    kv_norm_w = np.asarray(kv_norm_w, f)
    q_norm_w = np.asarray(q_norm_w, f)
    w_uk_e = np.asarray(w_uk, f) * kv_norm_w[:, None]
    w_ur_e = np.asarray(w_ur, f) * kv_norm_w[:, None]
    w_uv_e = np.asarray(w_uv, f) * kv_norm_w[:, None]
    w_uq_e = np.asarray(w_uq, f) * q_norm_w[:, None]
    w_qr_e = np.asarray(w_qr, f) * q_norm_w[:, None]
    w_o = np.asarray(w_o, f)

    wkv = np.ascontiguousarray(
        w_kv_down.reshape(NKC, KC, R).transpose(1, 0, 2).reshape(KC, D))
    wq = np.ascontiguousarray(
        w_q_down.reshape(NKC, KC, R).transpose(1, 0, 2).reshape(KC, D))
    cosT = np.ascontiguousarray(cos.T)                 # [32, S]
    sinT = np.ascontiguousarray(sin.T)
    sinSg = np.concatenate([-sinT[:DR // 2], sinT[DR // 2:]], axis=0)
    one32 = np.ones((DR, S), np.float32)
    zero32 = np.zeros((DR, S), np.float32)
    # pair-tensor rope tables: nope rows pass through (cos=1, sin=0)
    cosPt = np.concatenate([one32, cosT, one32, cosT], axis=0)   # [128, S]
    sinPt = np.concatenate([zero32, sinSg, zero32, sinSg], axis=0)
    # s-block interleave: [cos_blk0 | sin_blk0 | cos_blk1 | sin_blk1 | ...]
    cssin = np.empty((128, 2 * S), np.float32)
    for sb in range(NSB):
        cssin[:, sb * 2 * SB:sb * 2 * SB + SB] = \
            cosPt[:, sb * SB:(sb + 1) * SB]
        cssin[:, sb * 2 * SB + SB:(sb + 1) * 2 * SB] = \
            sinPt[:, sb * SB:(sb + 1) * SB]
    cssin = np.ascontiguousarray(cssin)
    # rope shift permutation within each head's 32 cols
    perm = np.concatenate([np.arange(16, 32), np.arange(0, 16)])

    # x: [b] -> transpose -> s-block-major [NSB, D, SB]
    xT4b = []
    for b in range(B):
        xT = x[b].T                                      # [D, S]
        xT4 = np.ascontiguousarray(
            xT.reshape(D, NSB, SB).transpose(1, 0, 2)).astype(md.bfloat16)
        xT4b.append(xT4)
    maskT8 = None
    if use_mask:
        m = np.asarray(mask, f).reshape(S, S)
        maskT8 = np.ascontiguousarray(m.T) * 8.0

    in_maps = []
    z32 = np.zeros((R, DN), np.float32)
    for core in range(NCORES):
        b, g = core // 4, core % 4
        cs = slice(g * NH * DN, (g + 1) * NH * DN)      # 128-wide col slice
        vs = slice(g * NH * HD, (g + 1) * NH * HD)      # 256-wide
        uk_l = w_uk_e[:, cs].reshape(R, NH, DN)
        ur_l = w_ur_e[:, cs].reshape(R, NH, DR)
        urs_l = ur_l[:, :, perm]
        uq_l = w_uq_e[:, cs].reshape(R, NH, DN)
        qr_l = w_qr_e[:, cs].reshape(R, NH, DR)
        qrs_l = qr_l[:, :, perm]

        def pair(nope, rope):
            cols = []
            for h in range(NH):
                cols += [nope[:, h], rope[:, h]]
            return np.ascontiguousarray(np.concatenate(cols, axis=1))

        def pair_sh(sh):
            cols = []
            for h in range(NH):
                cols += [z32, sh[:, h]]
            return np.ascontiguousarray(np.concatenate(cols, axis=1))

        wo_loc = w_o[g * NH * HD:(g + 1) * NH * HD]     # [256, D]
        wo_r = np.ascontiguousarray(
            wo_loc.reshape(2, KC, D).transpose(1, 0, 2).reshape(KC, 2 * D)
        ).astype(md.bfloat16)
        m_ = {
            "xT4": xT4b[b],
            "wkv": wkv.astype(md.bfloat16), "wq": wq.astype(md.bfloat16),
            "kb": pair(uk_l, ur_l).astype(md.bfloat16),
            "ksh": pair_sh(urs_l).astype(md.bfloat16),
            "qb": pair(uq_l, qr_l).astype(md.bfloat16),
            "qsh": pair_sh(qrs_l).astype(md.bfloat16),
            "uv": np.ascontiguousarray(w_uv_e[:, vs]).astype(md.bfloat16),
            "wo": wo_r,
            "cssin": cssin,
        }
        if use_mask:
            m_["maskT"] = maskT8
        in_maps.append(m_)
    return in_maps


def _classify_mask(mask):
    m = np.asarray(mask, np.float32).reshape(S, S)
    if not np.any(m):
        return False, False          # dense, no mask
    causal_ref = np.where(
        np.tril(np.ones((S, S), dtype=bool)), np.float32(0.0),
        np.float32(-1e9))
    if np.array_equal(m, causal_ref):
        return True, False           # structural causal
    return False, True               # generic additive mask


LAST_RESULTS = None


def kernel(**inputs):
    global LAST_RESULTS
    from concourse.bass_utils import run_bass_kernel_spmd
    causal, use_mask = _classify_mask(inputs["mask"])
    nc = _get_nc(causal, use_mask)
    in_maps = _prep_inputs(
        inputs["x"], inputs["cos"], inputs["sin"], inputs["mask"],
        inputs["w_kv_down"], inputs["kv_norm_w"], inputs["w_uk"],
        inputs["w_ur"], inputs["w_uv"], inputs["w_q_down"],
        inputs["q_norm_w"], inputs["w_uq"], inputs["w_qr"], inputs["w_o"],
        use_mask)
    res = run_bass_kernel_spmd(nc, in_maps, list(range(NCORES)))
    LAST_RESULTS = res
    parts = [np.asarray(res.results[c]["y"], np.float32)
             for c in range(NCORES)]
    out = np.empty((B, S, D), np.float32)
    for b in range(B):
        out[b] = parts[4 * b] + parts[4 * b + 1] + parts[4 * b + 2] \
            + parts[4 * b + 3]
    return out


# revision 23
# speedup vs baseline: 1.4000x; 1.0364x over previous
"""DeepSeek-MLA forward kernel for 8 Trainium2 NeuronCores (Bass/Tile).

Sharding: core c -> batch b = c // 4, head-group g = c % 4 (4 of 16 heads).
Each core computes its batch's down-projections, its 4 heads' attention,
and a partial output projection; the host sums the 4 partials per batch.

Design notes (v2, restructured from the 344us baseline):
- All of x is streamed to SBUF up front in per-s-block contiguous chunks;
  phase 1 (down-proj) runs as one uninterrupted PE stream.
- KV-side RMS norm is deferred: c_kv stays unnormalized; 1/rms(c_kv) enters
  as the per-partition (per-key) scale vector of the softmax exp and as the
  per-partition scale of the V tiles (additionally folded with the 1/8
  softmax temperature).  Q-side norm is applied in phase 1 via a
  DMA-broadcast row multiply.
- Norm statistics use PE ones-matmuls (row + column orientations) instead
  of slow gpsimd partition reduce/broadcast ops.
- Attention processes head PAIRS in [128,1024] two-bank PSUM tiles: one
  exp ACTIVATE per pair halves the ACT instruction count.  The softmax
  reciprocal runs on DVE (reciprocal_approx_fast) so the ACT engine never
  switches activation tables mid-stream.
- The output projection (P4) of block j-1 is emitted interleaved into the
  attention waves of block j as PE filler work, keeping the tensor engine
  continuously busy (p-state ramp to 2.4 GHz).
- y partials are stored fp16 (halves DMA); host sums in fp32.
"""

import os
import sys

import numpy as np

for _p in ("/opt/trn_rl_repo", "/root/.axon_site/_ro/trn_rl_repo"):
    if os.path.isdir(_p) and _p not in sys.path:
        sys.path.insert(0, _p)

import concourse.bass as bass
import concourse.mybir as mybir
import concourse.tile as tile
from concourse import bacc

B, S, D, H, DN, DR, R = 2, 2048, 2048, 16, 32, 32, 128
HD = DN + DR  # 64
EPS = 1e-5
NCORES = 8
NH = 4          # heads per core
SB = 512        # s-block (psum bank width in f32)
NSB = S // SB   # 4
ST = 128        # s-tile
NST = S // ST   # 16
KC = 128        # contraction chunk
NKC = D // KC   # 16
VW = HD + 1     # v columns incl. eighth column (65)
F32 = mybir.dt.float32
BF16 = mybir.dt.bfloat16
F16 = mybir.dt.float16
LN_EIGHTH = float(np.log(0.125))


def _build_nc(causal: bool, use_mask: bool):
    nc = bacc.Bacc("TRN2", target_bir_lowering=False, debug=False,
                   num_devices=NCORES)

    # x stored s-block-major: [NSB, D, SB] so each block's 16 row-chunks are
    # contiguous 128KB DMAs and phase 1 can start after ~1 chunk.
    xT4 = nc.dram_tensor("xT4", [NSB, D, SB], BF16, kind="ExternalInput").ap()
    wkv = nc.dram_tensor("wkv", [KC, D], BF16, kind="ExternalInput").ap()
    wq = nc.dram_tensor("wq", [KC, D], BF16, kind="ExternalInput").ap()
    kb = nc.dram_tensor("kb", [R, 2 * KC], BF16, kind="ExternalInput").ap()
    ksh = nc.dram_tensor("ksh", [R, 2 * KC], BF16, kind="ExternalInput").ap()
    qb = nc.dram_tensor("qb", [R, 2 * KC], BF16, kind="ExternalInput").ap()
    qsh = nc.dram_tensor("qsh", [R, 2 * KC], BF16, kind="ExternalInput").ap()
    uv = nc.dram_tensor("uv", [R, NH * HD], BF16, kind="ExternalInput").ap()
    wo = nc.dram_tensor("wo", [KC, 2 * D], BF16, kind="ExternalInput").ap()
    # rope tables, s-block interleaved: per sb, cols [0:512]=cos, [512:1024]=sin
    cssin = nc.dram_tensor("cssin", [128, 2 * S], F32, kind="ExternalInput").ap()
    maskT = None
    if use_mask:
        maskT = nc.dram_tensor("maskT", [S, S], F32, kind="ExternalInput").ap()
    y = nc.dram_tensor("y", [S, D], F16, kind="ExternalOutput").ap()
    dbg = {}
    if os.environ.get("MLA_DEBUG"):
        for nm, shp, dt in (("d_ckvT", [R, S], BF16), ("d_cqT", [R, S], BF16),
                            ("d_kT01", [128, S], BF16), ("d_kT23", [128, S], BF16),
                            ("d_qT01", [128, S], BF16), ("d_qT23", [128, S], BF16),
                            ("d_rstdq", [1, S], F32), ("d_rstdk8", [128, NST], F32),
                            ("d_vsb", [128, NST * NH * VW], BF16),
                            ("d_outT01", [128, S], BF16), ("d_outT23", [128, S], BF16),
                            ("d_den", [1, S], F32), ("d_rc", [1, S], F32)):
            dbg[nm] = nc.dram_tensor(nm, shp, dt, kind="ExternalOutput").ap()

    AF = mybir.ActivationFunctionType
    ALU = mybir.AluOpType

    with tile.TileContext(nc) as tc:
        from contextlib import ExitStack
        with ExitStack() as ctx:
            stat = ctx.enter_context(tc.tile_pool(name="static", bufs=1))
            # persistent SBUF tensors
            xall = stat.tile([128, NKC * S], BF16, name="xall")
            ckvT = stat.tile([R, S], BF16, name="ckvT")
            cqT = stat.tile([R, S], BF16, name="cqT")
            kT01 = stat.tile([128, S], BF16, name="kT01")
            kT23 = stat.tile([128, S], BF16, name="kT23")
            qT01 = stat.tile([128, S], BF16, name="qT01")
            qT23 = stat.tile([128, S], BF16, name="qT23")
            v_sb = stat.tile([128, NST * NH * VW], BF16, name="v_sb")
            outT01 = stat.tile([128, S], BF16, name="outT01")
            outT23 = stat.tile([128, S], BF16, name="outT23")
            wkv_sb = stat.tile([KC, D], BF16, name="wkv_sb")
            wq_sb = stat.tile([KC, D], BF16, name="wq_sb")
            kb_sb = stat.tile([R, 2 * KC], BF16, name="kb_sb")
            ksh_sb = stat.tile([R, 2 * KC], BF16, name="ksh_sb")
            qb_sb = stat.tile([R, 2 * KC], BF16, name="qb_sb")
            qsh_sb = stat.tile([R, 2 * KC], BF16, name="qsh_sb")
            uv_sb = stat.tile([R, NH * HD], BF16, name="uv_sb")
            wo_sb = stat.tile([KC, 2 * D], BF16, name="wo_sb")
            cssin_sb = stat.tile([128, 2 * S], F32, name="cssin_sb")
            tri_sb = stat.tile([128, 128], BF16, name="tri_sb")
            ones_col = stat.tile([128, 1], BF16, name="ones_col")
            rstdk8 = stat.tile([128, NST], F32, name="rstdk8")
            rstdq = stat.tile([1, S], F32, name="rstdq")
            rstdkv = stat.tile([1, S], F32, name="rstdkv") if use_mask else None
            msq_row = stat.tile([1, S], F32, name="msq_row")
            mskv_row = stat.tile([1, S], F32, name="mskv_row") if use_mask \
                else None
            msT_all = stat.tile([128, NST], F32, name="msT_all")
            eps_sb = stat.tile([128, 1], F32, name="eps_sb")
            dendump = stat.tile([1, S], F32, name="dendump") if dbg else None
            rcdump = stat.tile([1, S], F32, name="rcdump") if dbg else None
            ln8_sb = stat.tile([128, 1], F32, name="ln8_sb")

            # ---- static loads (wkv/wq first: phase 1 needs them) ----
            nc.sync.dma_start(wkv_sb[:], wkv)
            nc.sync.dma_start(wq_sb[:], wq)
            # x, s-block-major; one strided DMA per block (SP trigger cost)
            for sb in range(NSB):
                nc.sync.dma_start(
                    xall[:, sb * NKC * SB:(sb + 1) * NKC * SB].rearrange(
                        "p (k f) -> p k f", k=NKC),
                    xT4[sb].rearrange("(k p) f -> p k f", p=KC))
            nc.sync.dma_start(kb_sb[:], kb)
            nc.sync.dma_start(ksh_sb[:], ksh)
            nc.sync.dma_start(qb_sb[:], qb)
            nc.sync.dma_start(qsh_sb[:], qsh)
            nc.sync.dma_start(uv_sb[:], uv)
            nc.sync.dma_start(wo_sb[:], wo)
            nc.sync.dma_start(cssin_sb[:], cssin)

            nc.gpsimd.memset(ones_col[:], 1.0)
            nc.gpsimd.memset(eps_sb[:], EPS)
            nc.gpsimd.memset(ln8_sb[:], LN_EIGHTH)
            # v gets pre-filled with 0.125: the "denominator" column of each
            # head block stays 0.125 so PV accumulates denom/8 and the DVE
            # reciprocal directly yields 8/denom (the 8 un-does the 1/8
            # temperature folded into the V scale).
            nc.gpsimd.memset(v_sb[:], 0.125)
            nc.gpsimd.memset(tri_sb[:], 1.0)
            nc.gpsimd.affine_select(
                out=tri_sb[:], in_=tri_sb[:], compare_op=ALU.is_ge,
                fill=0.0, base=0, channel_multiplier=-1, pattern=[[1, 128]])

            v_blocks = v_sb.rearrange("p (t h w) -> p t h w", t=NST, h=NH)

            # pools
            pair = ctx.enter_context(
                tc.tile_pool(name="pair", bufs=2, space="PSUM"))
            oaccp = ctx.enter_context(
                tc.tile_pool(name="oaccp", bufs=4, space="PSUM"))
            sqp = ctx.enter_context(tc.tile_pool(name="sqp", bufs=2))
            t12p = ctx.enter_context(tc.tile_pool(name="t12p", bufs=1 if use_mask else 2))
            etp = ctx.enter_context(tc.tile_pool(name="etp", bufs=3 if use_mask else 4))
            rcp = ctx.enter_context(tc.tile_pool(name="rcp", bufs=2 if use_mask else 3))
            tinyp = ctx.enter_context(tc.tile_pool(name="tinyp", bufs=2))
            rbqp = ctx.enter_context(tc.tile_pool(name="rbqp", bufs=2))
            rbop = ctx.enter_context(tc.tile_pool(name="rbop", bufs=2 if use_mask else 3))
            ysbp = ctx.enter_context(tc.tile_pool(name="ysbp", bufs=2 if use_mask else 3))
            tailp = None if use_mask else ctx.enter_context(tc.tile_pool(name="tailp", bufs=1))
            mtp = ctx.enter_context(tc.tile_pool(name="mtp", bufs=2)) \
                if use_mask else None

            # ================= Phase 1: down-projections + norms ==========
            # cT tiles stay UNNORMALIZED; norm stats are staged to SBUF so
            # all Ln/Exp activations run as one batched pair (2 table loads).
            # rstd_q is applied post-rope in P2; rstd_kv/8 rides the exp
            # scale vector + V tile scale.
            for sb in range(NSB):
                sl = slice(sb * SB, (sb + 1) * SB)
                cps = pair.tile([128, 2 * SB], F32, name="cps", tag="pair")
                for k in range(NKC):
                    xsl = xall[:, (sb * NKC + k) * SB:(sb * NKC + k + 1) * SB]
                    nc.tensor.matmul(cps[:, 0:SB],
                                     wkv_sb[:, k * KC:(k + 1) * KC], xsl,
                                     start=(k == 0), stop=(k == NKC - 1))
                    nc.tensor.matmul(cps[:, SB:2 * SB],
                                     wq_sb[:, k * KC:(k + 1) * KC], xsl,
                                     start=(k == 0), stop=(k == NKC - 1))
                sq = sqp.tile([128, 2 * SB], BF16, name="sq", tag="sq")
                nc.scalar.activation(sq[:], cps[:], AF.Square)
                small = pair.tile([128, 2 * SB], F32, name="small", tag="pair")
                # q-side row sums: [1, 512]
                nc.tensor.matmul(small[0:1, 0:SB], ones_col[:], sq[:, SB:2 * SB])
                if use_mask:
                    nc.tensor.matmul(small[32:33, 0:SB], ones_col[:], sq[:, 0:SB])
                # kv-side column sums: [128, 1] per s-tile
                for t in range(4):
                    nc.tensor.matmul(small[:, SB + t:SB + t + 1],
                                     sq[:, t * ST:(t + 1) * ST], ones_col[:])
                # stage stats to SBUF on ACT (Copy lives in every act table)
                nc.scalar.activation(msq_row[0:1, sl], small[0:1, 0:SB],
                                     AF.Copy)
                if use_mask:
                    nc.scalar.activation(mskv_row[0:1, sl],
                                         small[32:33, 0:SB], AF.Copy)
                nc.scalar.activation(msT_all[:, sb * 4:(sb + 1) * 4],
                                     small[:, SB:SB + 4], AF.Copy)
                nc.vector.tensor_copy(ckvT[:, sl], cps[:, 0:SB])
                nc.vector.tensor_copy(cqT[:, sl], cps[:, SB:2 * SB])
            # batched norm statistics: one Ln set load + one Exp set load
            if not use_mask:
                tq = tailp.tile([1, S], F32, name="tq", tag="tail")
                nc.scalar.activation(tq[:], msq_row[:], AF.Ln,
                                     bias=eps_sb[0:1, :], scale=1.0 / R)
                nc.scalar.activation(rstdq[:], tq[:], AF.Exp, scale=-0.5)
            else:
                # chunked through small tiles (mask path is SBUF-tight)
                for row_src, row_dst in ((msq_row, rstdq),
                                         (mskv_row, rstdkv)):
                    for c in range(NSB):
                        cl = slice(c * SB, (c + 1) * SB)
                        t = rcp.tile([1, SB], F32, name="tln", tag="rc")
                        nc.scalar.activation(t[:], row_src[0:1, cl], AF.Ln,
                                             bias=eps_sb[0:1, :],
                                             scale=1.0 / R)
                        nc.scalar.activation(row_dst[0:1, cl], t[:], AF.Exp,
                                             scale=-0.5)
            tk = tinyp.tile([128, NST], F32, name="tk", tag="tiny")
            nc.scalar.activation(tk[:], msT_all[:], AF.Ln,
                                 bias=eps_sb[:], scale=1.0 / R)
            nc.scalar.activation(rstdk8[:], tk[:], AF.Exp,
                                 scale=-0.5, bias=ln8_sb[:])

            # ================= Fused P2 -> P3 -> P4 pipeline ==============
            def emit_p2(sb):
                sl = slice(sb * SB, (sb + 1) * SB)
                cs_sl = cssin_sb[:, sb * 2 * SB:(sb + 1) * 2 * SB]
                rbq = rbqp.tile([128, SB], F32, name="rbq", tag="rbq")
                nc.gpsimd.partition_broadcast(rbq[:], rstdq[0:1, sl])
                rbkv = None
                if use_mask:
                    rbkv = rbqp.tile([128, SB], F32, name="rbkv", tag="rbq")
                    nc.gpsimd.partition_broadcast(rbkv[:], rstdkv[0:1, sl])
                for cT, wb, wsh, dsts, rb in (
                        (ckvT, kb_sb, ksh_sb, (kT01, kT23), rbkv),
                        (cqT, qb_sb, qsh_sb, (qT01, qT23), rbq)):
                    for p in range(2):
                        pp = pair.tile([128, 2 * SB], F32, name="pp",
                                       tag="pair")
                        nc.tensor.matmul(pp[:, 0:SB],
                                         wb[:, p * KC:(p + 1) * KC], cT[:, sl])
                        nc.tensor.matmul(pp[:, SB:2 * SB],
                                         wsh[:, p * KC:(p + 1) * KC], cT[:, sl])
                        t12 = t12p.tile([128, 2 * SB], F32, name="t12",
                                        tag="t12")
                        nc.vector.tensor_mul(t12[:], pp[:], cs_sl)
                        nc.vector.tensor_add(dsts[p][:, sl], t12[:, 0:SB],
                                             t12[:, SB:2 * SB])
                        if rb is not None:
                            # apply the deferred per-position rstd in place
                            nc.vector.tensor_mul(dsts[p][:, sl],
                                                 dsts[p][:, sl], rb[:])
                # v tiles for this block's 4 s-tiles
                vps = pair.tile([128, 2 * SB], F32, name="vps", tag="pair")
                for t in range(4):
                    g = sb * 4 + t
                    nc.tensor.matmul(vps[:, t * 256:(t + 1) * 256],
                                     ckvT[:, g * ST:(g + 1) * ST], uv_sb[:])
                for t in range(4):
                    g = sb * 4 + t
                    src = vps[:, t * 256:(t + 1) * 256].rearrange(
                        "p (h d) -> p h d", h=NH)
                    nc.vector.tensor_scalar_mul(
                        v_blocks[:, g, :, 0:HD], src, rstdk8[:, g:g + 1])

            def p4_units(j):
                units = []
                for t in range(4 * j, 4 * j + 4):
                    for dh in range(2):
                        def emit(t=t, dh=dh):
                            yp = pair.tile([128, 2 * SB], F32, name="yp",
                                           tag="pair")
                            for di in range(2):
                                dcol = dh * 1024 + di * SB
                                for c, oT in ((0, outT01), (1, outT23)):
                                    nc.tensor.matmul(
                                        yp[:, di * SB:(di + 1) * SB],
                                        oT[:, t * ST:(t + 1) * ST],
                                        wo_sb[:, c * D + dcol:c * D + dcol + SB],
                                        start=(c == 0), stop=(c == 1))
                            ysb = ysbp.tile([128, 2 * SB], F16, name="ysb",
                                            tag="ysb")
                            if dh == 0:
                                nc.scalar.activation(ysb[:], yp[:], AF.Copy)
                            else:
                                nc.vector.tensor_copy(ysb[:], yp[:])
                            nc.sync.dma_start(
                                y[t * ST:(t + 1) * ST,
                                  dh * 1024:(dh + 1) * 1024], ysb[:])
                        units.append(emit)
                return units

            def emit_scores(j, i):
                """Emit score matmuls + exp (+mask/tri) for key-tile i of
                block j, both head pairs. Returns the et tiles."""
                q0 = ST * (i - 4 * j) if (causal and i >= 4 * j) else 0
                ets = []
                mt = None
                if use_mask:
                    mt = mtp.tile([128, SB], F32, name="mt", tag="mt")
                    nc.sync.dma_start(
                        mt[:], maskT[i * ST:(i + 1) * ST,
                                     j * SB:(j + 1) * SB])
                for hp, (kTp, qTp) in enumerate(((kT01, qT01), (kT23, qT23))):
                    sc2 = pair.tile([128, 2 * SB], F32, name="sc2", tag="pair")
                    for half in range(2):
                        hs = slice(half * 64, half * 64 + 64)
                        nc.tensor.matmul(
                            sc2[:, half * SB + q0:(half + 1) * SB],
                            kTp[hs, i * ST:(i + 1) * ST],
                            qTp[hs, j * SB + q0:(j + 1) * SB])
                    if use_mask:
                        for half in range(2):
                            nc.vector.tensor_add(
                                sc2[:, half * SB:(half + 1) * SB],
                                sc2[:, half * SB:(half + 1) * SB], mt[:])
                    et = etp.tile([128, 2 * SB], BF16, name="et", tag="et")
                    scal = 0.125 if use_mask else rstdk8[:, i:i + 1]
                    if q0 == 0:
                        nc.scalar.activation(et[:], sc2[:], AF.Exp, scale=scal)
                    else:
                        for half in range(2):
                            rg = slice(half * SB + q0, (half + 1) * SB)
                            nc.scalar.activation(et[:, rg], sc2[:, rg],
                                                 AF.Exp, scale=scal)
                    if causal and i >= 4 * j:
                        for half in range(2):
                            rg = slice(half * SB + q0, half * SB + q0 + ST)
                            nc.vector.tensor_mul(et[:, rg], et[:, rg], tri_sb[:])
                    ets.append((et, q0))
                return ets

            def emit_pv(j, i, ets, first, last):
                for hp, (et, q0) in enumerate(ets):
                    for half in range(2):
                        h = 2 * hp + half
                        nc.tensor.matmul(
                            oacc[h][:, q0:SB],
                            v_sb[:, i * (NH * VW) + h * VW:
                                 i * (NH * VW) + (h + 1) * VW],
                            et[:, half * SB + q0:(half + 1) * SB],
                            start=first, stop=last)

            for j in range(NSB):
                emit_p2(j)
                ktiles = list(range(4 * j + 4)) if causal else list(range(NST))
                oacc = [oaccp.tile([VW, SB], F32, name=f"oa{h}", tag="oa")
                        for h in range(NH)]
                units = p4_units(j - 1) if j > 0 else []
                ui = 0
                prev = None
                for i in ktiles:
                    ets = emit_scores(j, i)
                    if ui < len(units):
                        units[ui]()
                        ui += 1
                    if prev is not None:
                        emit_pv(j, prev[0], prev[1],
                                prev[0] == ktiles[0], False)
                    prev = (i, ets)
                emit_pv(j, prev[0], prev[1], prev[0] == ktiles[0], True)
                while ui < len(units):
                    units[ui]()
                    ui += 1
                # block tail: softmax denominators + output scaling
                for h in range(NH):
                    den = rcp.tile([1, SB], F32, name="den", tag="rc")
                    nc.vector.tensor_copy(den[:], oacc[h][HD:VW, :])
                    rc = rcp.tile([1, SB], F32, name="rc", tag="rc")
                    nc.vector.reciprocal_approx_fast(out=rc[:], in_=den[:])
                    if dbg and h < 1:
                        o0 = j * SB
                        nc.vector.tensor_copy(
                            dendump[0:1, o0:o0 + SB], oacc[h][HD:VW, :])
                        nc.vector.tensor_copy(
                            rcdump[0:1, o0:o0 + SB], rc[:])
                    rbo = rbop.tile([HD, SB], F32, name="rbo", tag="rbo")
                    nc.gpsimd.partition_broadcast(rbo[:], rc[:])
                    dst = (outT01 if h < 2 else outT23)[
                        (h % 2) * HD:(h % 2 + 1) * HD,
                        j * SB:(j + 1) * SB]
                    nc.vector.tensor_mul(dst, oacc[h][0:HD, :], rbo[:])
            for u in p4_units(NSB - 1):
                u()
            if dbg:
                for nm, t in (("d_ckvT", ckvT), ("d_cqT", cqT),
                              ("d_kT01", kT01), ("d_kT23", kT23),
                              ("d_qT01", qT01), ("d_qT23", qT23),
                              ("d_rstdq", rstdq), ("d_rstdk8", rstdk8),
                              ("d_vsb", v_sb),
                              ("d_outT01", outT01), ("d_outT23", outT23),
                              ("d_den", dendump), ("d_rc", rcdump)):
                    nc.sync.dma_start(dbg[nm], t[:])

    nc.finalize()
    return nc


_NC_CACHE = {}


def _get_nc(causal, use_mask):
    key = (causal, use_mask)
    if key not in _NC_CACHE:
        _NC_CACHE[key] = _build_nc(causal, use_mask)
    return _NC_CACHE[key]


def _prep_inputs(x, cos, sin, mask, w_kv_down, kv_norm_w, w_uk, w_ur, w_uv,
                 w_q_down, q_norm_w, w_uq, w_qr, w_o, use_mask):
    """Build the 8 per-core input maps (host-side shard + fold)."""
    import ml_dtypes as md
    f = np.float32
    x = np.asarray(x, f)
    cos = np.asarray(cos, f)
    sin = np.asarray(sin, f)
    w_kv_down = np.asarray(w_kv_down, f)
    w_q_down = np.asarray(w_q_down, f)
    kv_norm_w = np.asarray(kv_norm_w, f)
    q_norm_w = np.asarray(q_norm_w, f)
    w_uk_e = np.asarray(w_uk, f) * kv_norm_w[:, None]
    w_ur_e = np.asarray(w_ur, f) * kv_norm_w[:, None]
    w_uv_e = np.asarray(w_uv, f) * kv_norm_w[:, None]
    w_uq_e = np.asarray(w_uq, f) * q_norm_w[:, None]
    w_qr_e = np.asarray(w_qr, f) * q_norm_w[:, None]
    w_o = np.asarray(w_o, f)

    wkv = np.ascontiguousarray(
        w_kv_down.reshape(NKC, KC, R).transpose(1, 0, 2).reshape(KC, D))
    wq = np.ascontiguousarray(
        w_q_down.reshape(NKC, KC, R).transpose(1, 0, 2).reshape(KC, D))
    cosT = np.ascontiguousarray(cos.T)                 # [32, S]
    sinT = np.ascontiguousarray(sin.T)
    sinSg = np.concatenate([-sinT[:DR // 2], sinT[DR // 2:]], axis=0)
    one32 = np.ones((DR, S), np.float32)
    zero32 = np.zeros((DR, S), np.float32)
    # pair-tensor rope tables: nope rows pass through (cos=1, sin=0)
    cosPt = np.concatenate([one32, cosT, one32, cosT], axis=0)   # [128, S]
    sinPt = np.concatenate([zero32, sinSg, zero32, sinSg], axis=0)
    # s-block interleave: [cos_blk0 | sin_blk0 | cos_blk1 | sin_blk1 | ...]
    cssin = np.empty((128, 2 * S), np.float32)
    for sb in range(NSB):
        cssin[:, sb * 2 * SB:sb * 2 * SB + SB] = \
            cosPt[:, sb * SB:(sb + 1) * SB]
        cssin[:, sb * 2 * SB + SB:(sb + 1) * 2 * SB] = \
            sinPt[:, sb * SB:(sb + 1) * SB]
    cssin = np.ascontiguousarray(cssin)
    # rope shift permutation within each head's 32 cols
    perm = np.concatenate([np.arange(16, 32), np.arange(0, 16)])

    # x: [b] -> transpose -> s-block-major [NSB, D, SB]
    xT4b = []
    for b in range(B):
        xT = x[b].T                                      # [D, S]
        xT4 = np.ascontiguousarray(
            xT.reshape(D, NSB, SB).transpose(1, 0, 2)).astype(md.bfloat16)
        xT4b.append(xT4)
    maskT8 = None
    if use_mask:
        m = np.asarray(mask, f).reshape(S, S)
        maskT8 = np.ascontiguousarray(m.T) * 8.0

    in_maps = []
    z32 = np.zeros((R, DN), np.float32)
    for core in range(NCORES):
        b, g = core // 4, core % 4
        cs = slice(g * NH * DN, (g + 1) * NH * DN)      # 128-wide col slice
        vs = slice(g * NH * HD, (g + 1) * NH * HD)      # 256-wide
        uk_l = w_uk_e[:, cs].reshape(R, NH, DN)
        ur_l = w_ur_e[:, cs].reshape(R, NH, DR)
        urs_l = ur_l[:, :, perm]
        uq_l = w_uq_e[:, cs].reshape(R, NH, DN)
        qr_l = w_qr_e[:, cs].reshape(R, NH, DR)
        qrs_l = qr_l[:, :, perm]

        def pair(nope, rope):
            cols = []
            for h in range(NH):
                cols += [nope[:, h], rope[:, h]]
            return np.ascontiguousarray(np.concatenate(cols, axis=1))

        def pair_sh(sh):
            cols = []
            for h in range(NH):
                cols += [z32, sh[:, h]]
            return np.ascontiguousarray(np.concatenate(cols, axis=1))

        wo_loc = w_o[g * NH * HD:(g + 1) * NH * HD]     # [256, D]
        wo_r = np.ascontiguousarray(
            wo_loc.reshape(2, KC, D).transpose(1, 0, 2).reshape(KC, 2 * D)
        ).astype(md.bfloat16)
        m_ = {
            "xT4": xT4b[b],
            "wkv": wkv.astype(md.bfloat16), "wq": wq.astype(md.bfloat16),
            "kb": pair(uk_l, ur_l).astype(md.bfloat16),
            "ksh": pair_sh(urs_l).astype(md.bfloat16),
            "qb": pair(uq_l, qr_l).astype(md.bfloat16),
            "qsh": pair_sh(qrs_l).astype(md.bfloat16),
            "uv": np.ascontiguousarray(w_uv_e[:, vs]).astype(md.bfloat16),
            "wo": wo_r,
            "cssin": cssin,
        }
        if use_mask:
            m_["maskT"] = maskT8
        in_maps.append(m_)
    return in_maps


def _classify_mask(mask):
    m = np.asarray(mask, np.float32).reshape(S, S)
    if not np.any(m):
        return False, False          # dense, no mask
    causal_ref = np.where(
        np.tril(np.ones((S, S), dtype=bool)), np.float32(0.0),
        np.float32(-1e9))
    if np.array_equal(m, causal_ref):
        return True, False           # structural causal
    return False, True               # generic additive mask


LAST_RESULTS = None


def kernel(**inputs):
    global LAST_RESULTS
    from concourse.bass_utils import run_bass_kernel_spmd
    causal, use_mask = _classify_mask(inputs["mask"])
    nc = _get_nc(causal, use_mask)
    in_maps = _prep_inputs(
        inputs["x"], inputs["cos"], inputs["sin"], inputs["mask"],
        inputs["w_kv_down"], inputs["kv_norm_w"], inputs["w_uk"],
        inputs["w_ur"], inputs["w_uv"], inputs["w_q_down"],
        inputs["q_norm_w"], inputs["w_uq"], inputs["w_qr"], inputs["w_o"],
        use_mask)
    res = run_bass_kernel_spmd(nc, in_maps, list(range(NCORES)))
    LAST_RESULTS = res
    parts = [np.asarray(res.results[c]["y"], np.float32)
             for c in range(NCORES)]
    out = np.empty((B, S, D), np.float32)
    for b in range(B):
        out[b] = parts[4 * b] + parts[4 * b + 1] + parts[4 * b + 2] \
            + parts[4 * b + 3]
    return out


# revision 24
# speedup vs baseline: 1.4925x; 1.0661x over previous
"""DeepSeek-MLA forward kernel for 8 Trainium2 NeuronCores (Bass/Tile).

Sharding: core c -> batch b = c // 4, head-group g = c % 4 (4 of 16 heads).
Each core computes its batch's down-projections, its 4 heads' attention,
and a partial output projection; the host sums the 4 partials per batch.

Design notes (v2, restructured from the 344us baseline):
- All of x is streamed to SBUF up front in per-s-block contiguous chunks;
  phase 1 (down-proj) runs as one uninterrupted PE stream.
- KV-side RMS norm is deferred: c_kv stays unnormalized; 1/rms(c_kv) enters
  as the per-partition (per-key) scale vector of the softmax exp and as the
  per-partition scale of the V tiles (additionally folded with the 1/8
  softmax temperature).  Q-side norm is applied in phase 1 via a
  DMA-broadcast row multiply.
- Norm statistics use PE ones-matmuls (row + column orientations) instead
  of slow gpsimd partition reduce/broadcast ops.
- Attention processes head PAIRS in [128,1024] two-bank PSUM tiles: one
  exp ACTIVATE per pair halves the ACT instruction count.  The softmax
  reciprocal runs on DVE (reciprocal_approx_fast) so the ACT engine never
  switches activation tables mid-stream.
- The output projection (P4) of block j-1 is emitted interleaved into the
  attention waves of block j as PE filler work, keeping the tensor engine
  continuously busy (p-state ramp to 2.4 GHz).
- y partials are stored fp16 (halves DMA); host sums in fp32.
"""

import os
import sys

import numpy as np

for _p in ("/opt/trn_rl_repo", "/root/.axon_site/_ro/trn_rl_repo"):
    if os.path.isdir(_p) and _p not in sys.path:
        sys.path.insert(0, _p)

import concourse.bass as bass
import concourse.mybir as mybir
import concourse.tile as tile
from concourse import bacc

B, S, D, H, DN, DR, R = 2, 2048, 2048, 16, 32, 32, 128
HD = DN + DR  # 64
EPS = 1e-5
NCORES = 8
NH = 4          # heads per core
SB = 512        # s-block (psum bank width in f32)
NSB = S // SB   # 4
ST = 128        # s-tile
NST = S // ST   # 16
KC = 128        # contraction chunk
NKC = D // KC   # 16
VW = HD + 1     # v columns incl. eighth column (65)
F32 = mybir.dt.float32
BF16 = mybir.dt.bfloat16
F16 = mybir.dt.float16
LN_EIGHTH = float(np.log(0.125))


def _build_nc(causal: bool, use_mask: bool):
    nc = bacc.Bacc("TRN2", target_bir_lowering=False, debug=False,
                   num_devices=NCORES)

    # x stored s-block-major: [NSB, D, SB] so each block's 16 row-chunks are
    # contiguous 128KB DMAs and phase 1 can start after ~1 chunk.
    xT4 = nc.dram_tensor("xT4", [NSB, D, SB], BF16, kind="ExternalInput").ap()
    wkv = nc.dram_tensor("wkv", [KC, D], BF16, kind="ExternalInput").ap()
    wq = nc.dram_tensor("wq", [KC, D], BF16, kind="ExternalInput").ap()
    kb = nc.dram_tensor("kb", [R, 2 * KC], BF16, kind="ExternalInput").ap()
    ksh = nc.dram_tensor("ksh", [R, 2 * KC], BF16, kind="ExternalInput").ap()
    qb = nc.dram_tensor("qb", [R, 2 * KC], BF16, kind="ExternalInput").ap()
    qsh = nc.dram_tensor("qsh", [R, 2 * KC], BF16, kind="ExternalInput").ap()
    uv = nc.dram_tensor("uv", [R, NH * HD], BF16, kind="ExternalInput").ap()
    wo = nc.dram_tensor("wo", [KC, 2 * D], BF16, kind="ExternalInput").ap()
    # rope tables, s-block interleaved: per sb, cols [0:512]=cos, [512:1024]=sin
    cssin = nc.dram_tensor("cssin", [128, 2 * S], F32, kind="ExternalInput").ap()
    maskT = None
    if use_mask:
        maskT = nc.dram_tensor("maskT", [S, S], F32, kind="ExternalInput").ap()
    y = nc.dram_tensor("y", [S, D], F16, kind="ExternalOutput").ap()
    dbg = {}
    if os.environ.get("MLA_DEBUG"):
        for nm, shp, dt in (("d_ckvT", [R, S], BF16), ("d_cqT", [R, S], BF16),
                            ("d_kT01", [128, S], BF16), ("d_kT23", [128, S], BF16),
                            ("d_qT01", [128, S], BF16), ("d_qT23", [128, S], BF16),
                            ("d_rstdq", [1, S], F32), ("d_rstdk8", [128, NST], F32),
                            ("d_vsb", [128, NST * NH * VW], BF16),
                            ("d_outT01", [128, S], BF16), ("d_outT23", [128, S], BF16),
                            ("d_den", [1, S], F32), ("d_rc", [1, S], F32)):
            dbg[nm] = nc.dram_tensor(nm, shp, dt, kind="ExternalOutput").ap()

    AF = mybir.ActivationFunctionType
    ALU = mybir.AluOpType

    with tile.TileContext(nc) as tc:
        from contextlib import ExitStack
        with ExitStack() as ctx:
            stat = ctx.enter_context(tc.tile_pool(name="static", bufs=1))
            # persistent SBUF tensors
            xall = stat.tile([128, NKC * S], BF16, name="xall")
            ckvT = stat.tile([R, S], BF16, name="ckvT")
            cqT = stat.tile([R, S], BF16, name="cqT")
            kT01 = stat.tile([128, S], BF16, name="kT01")
            kT23 = stat.tile([128, S], BF16, name="kT23")
            qT01 = stat.tile([128, S], BF16, name="qT01")
            qT23 = stat.tile([128, S], BF16, name="qT23")
            v_sb = stat.tile([128, NST * NH * VW], BF16, name="v_sb")
            outT01 = stat.tile([128, S], BF16, name="outT01")
            outT23 = stat.tile([128, S], BF16, name="outT23")
            wkv_sb = stat.tile([KC, D], BF16, name="wkv_sb")
            wq_sb = stat.tile([KC, D], BF16, name="wq_sb")
            kb_sb = stat.tile([R, 2 * KC], BF16, name="kb_sb")
            ksh_sb = stat.tile([R, 2 * KC], BF16, name="ksh_sb")
            qb_sb = stat.tile([R, 2 * KC], BF16, name="qb_sb")
            qsh_sb = stat.tile([R, 2 * KC], BF16, name="qsh_sb")
            uv_sb = stat.tile([R, NH * HD], BF16, name="uv_sb")
            wo_sb = stat.tile([KC, 2 * D], BF16, name="wo_sb")
            cssin_sb = stat.tile([128, 2 * S], F32, name="cssin_sb")
            tri_sb = stat.tile([128, 128], BF16, name="tri_sb")
            ones_col = stat.tile([128, 1], BF16, name="ones_col")
            rstdk8 = stat.tile([128, NST], F32, name="rstdk8")
            rstdq = stat.tile([1, S], F32, name="rstdq")
            rstdkv = stat.tile([1, S], F32, name="rstdkv") if use_mask else None
            msq_row = stat.tile([1, S], F32, name="msq_row")
            mskv_row = stat.tile([1, S], F32, name="mskv_row") if use_mask \
                else None
            msT_all = stat.tile([128, NST], F32, name="msT_all")
            eps_sb = stat.tile([128, 1], F32, name="eps_sb")
            dendump = stat.tile([1, S], F32, name="dendump") if dbg else None
            rcdump = stat.tile([1, S], F32, name="rcdump") if dbg else None
            ln8_sb = stat.tile([128, 1], F32, name="ln8_sb")

            # ---- static loads (wkv/wq first: phase 1 needs them) ----
            nc.sync.dma_start(wkv_sb[:], wkv)
            nc.sync.dma_start(wq_sb[:], wq)
            # x, s-block-major contiguous 128KB chunks
            for sb in range(NSB):
                for k in range(NKC):
                    nc.sync.dma_start(
                        xall[:, (sb * NKC + k) * SB:(sb * NKC + k + 1) * SB],
                        xT4[sb, k * KC:(k + 1) * KC, :])
            nc.sync.dma_start(kb_sb[:], kb)
            nc.sync.dma_start(ksh_sb[:], ksh)
            nc.sync.dma_start(qb_sb[:], qb)
            nc.sync.dma_start(qsh_sb[:], qsh)
            nc.sync.dma_start(uv_sb[:], uv)
            nc.sync.dma_start(wo_sb[:], wo)
            nc.sync.dma_start(cssin_sb[:], cssin)

            nc.gpsimd.memset(ones_col[:], 1.0)
            nc.gpsimd.memset(eps_sb[:], EPS)
            nc.gpsimd.memset(ln8_sb[:], LN_EIGHTH)
            # v gets pre-filled with 0.125: the "denominator" column of each
            # head block stays 0.125 so PV accumulates denom/8 and the DVE
            # reciprocal directly yields 8/denom (the 8 un-does the 1/8
            # temperature folded into the V scale).
            nc.gpsimd.memset(v_sb[:], 0.125)
            nc.gpsimd.memset(tri_sb[:], 1.0)
            nc.gpsimd.affine_select(
                out=tri_sb[:], in_=tri_sb[:], compare_op=ALU.is_ge,
                fill=0.0, base=0, channel_multiplier=-1, pattern=[[1, 128]])

            v_blocks = v_sb.rearrange("p (t h w) -> p t h w", t=NST, h=NH)

            # pools
            pair = ctx.enter_context(
                tc.tile_pool(name="pair", bufs=2, space="PSUM"))
            oaccp = ctx.enter_context(
                tc.tile_pool(name="oaccp", bufs=4, space="PSUM"))
            sqp = ctx.enter_context(tc.tile_pool(name="sqp", bufs=2))
            t12p = ctx.enter_context(tc.tile_pool(name="t12p", bufs=1 if use_mask else 2))
            etp = ctx.enter_context(tc.tile_pool(name="etp", bufs=3 if use_mask else 4))
            rcp = ctx.enter_context(tc.tile_pool(name="rcp", bufs=2 if use_mask else 3))
            tinyp = ctx.enter_context(tc.tile_pool(name="tinyp", bufs=2))
            rbqp = ctx.enter_context(tc.tile_pool(name="rbqp", bufs=2))
            rbop = ctx.enter_context(tc.tile_pool(name="rbop", bufs=2 if use_mask else 3))
            ysbp = ctx.enter_context(tc.tile_pool(name="ysbp", bufs=2 if use_mask else 3))
            tailp = None if use_mask else ctx.enter_context(tc.tile_pool(name="tailp", bufs=1))
            mtp = ctx.enter_context(tc.tile_pool(name="mtp", bufs=2)) \
                if use_mask else None

            # ================= Phase 1: down-projections + norms ==========
            # cT tiles stay UNNORMALIZED; norm stats are staged to SBUF so
            # all Ln/Exp activations run as one batched pair (2 table loads).
            # rstd_q is applied post-rope in P2; rstd_kv/8 rides the exp
            # scale vector + V tile scale.
            for sb in range(NSB):
                sl = slice(sb * SB, (sb + 1) * SB)
                cps = pair.tile([128, 2 * SB], F32, name="cps", tag="pair")
                for k in range(NKC):
                    xsl = xall[:, (sb * NKC + k) * SB:(sb * NKC + k + 1) * SB]
                    nc.tensor.matmul(cps[:, 0:SB],
                                     wkv_sb[:, k * KC:(k + 1) * KC], xsl,
                                     start=(k == 0), stop=(k == NKC - 1))
                    nc.tensor.matmul(cps[:, SB:2 * SB],
                                     wq_sb[:, k * KC:(k + 1) * KC], xsl,
                                     start=(k == 0), stop=(k == NKC - 1))
                sq = sqp.tile([128, 2 * SB], BF16, name="sq", tag="sq")
                nc.scalar.activation(sq[:], cps[:], AF.Square)
                small = pair.tile([128, 2 * SB], F32, name="small", tag="pair")
                # q-side row sums: [1, 512]
                nc.tensor.matmul(small[0:1, 0:SB], ones_col[:], sq[:, SB:2 * SB])
                if use_mask:
                    nc.tensor.matmul(small[32:33, 0:SB], ones_col[:], sq[:, 0:SB])
                # kv-side column sums: [128, 1] per s-tile
                for t in range(4):
                    nc.tensor.matmul(small[:, SB + t:SB + t + 1],
                                     sq[:, t * ST:(t + 1) * ST], ones_col[:])
                # stage stats to SBUF on ACT (Copy lives in every act table)
                nc.scalar.activation(msq_row[0:1, sl], small[0:1, 0:SB],
                                     AF.Copy)
                if use_mask:
                    nc.scalar.activation(mskv_row[0:1, sl],
                                         small[32:33, 0:SB], AF.Copy)
                nc.scalar.activation(msT_all[:, sb * 4:(sb + 1) * 4],
                                     small[:, SB:SB + 4], AF.Copy)
                nc.vector.tensor_copy(ckvT[:, sl], cps[:, 0:SB])
                nc.vector.tensor_copy(cqT[:, sl], cps[:, SB:2 * SB])
            # batched norm statistics: one Ln set load + one Exp set load
            if not use_mask:
                tq = tailp.tile([1, S], F32, name="tq", tag="tail")
                nc.scalar.activation(tq[:], msq_row[:], AF.Ln,
                                     bias=eps_sb[0:1, :], scale=1.0 / R)
                nc.scalar.activation(rstdq[:], tq[:], AF.Exp, scale=-0.5)
            else:
                # chunked through small tiles (mask path is SBUF-tight)
                for row_src, row_dst in ((msq_row, rstdq),
                                         (mskv_row, rstdkv)):
                    for c in range(NSB):
                        cl = slice(c * SB, (c + 1) * SB)
                        t = rcp.tile([1, SB], F32, name="tln", tag="rc")
                        nc.scalar.activation(t[:], row_src[0:1, cl], AF.Ln,
                                             bias=eps_sb[0:1, :],
                                             scale=1.0 / R)
                        nc.scalar.activation(row_dst[0:1, cl], t[:], AF.Exp,
                                             scale=-0.5)
            tk = tinyp.tile([128, NST], F32, name="tk", tag="tiny")
            nc.scalar.activation(tk[:], msT_all[:], AF.Ln,
                                 bias=eps_sb[:], scale=1.0 / R)
            nc.scalar.activation(rstdk8[:], tk[:], AF.Exp,
                                 scale=-0.5, bias=ln8_sb[:])

            # ================= Fused P2 -> P3 -> P4 pipeline ==============
            def p2_chunks(sb):
                """Up-projection + rope for block sb as filler closures, so
                block sb's kT/qT/v are produced during block sb-1's waves."""
                sl = slice(sb * SB, (sb + 1) * SB)
                cs_sl = cssin_sb[:, sb * 2 * SB:(sb + 1) * 2 * SB]
                state = {}

                def bcast():
                    rbq = rbqp.tile([128, SB], F32, name="rbq", tag="rbq")
                    nc.gpsimd.partition_broadcast(rbq[:], rstdq[0:1, sl])
                    state["rbq"] = rbq
                    if use_mask:
                        rbkv = rbqp.tile([128, SB], F32, name="rbkv",
                                         tag="rbq")
                        nc.gpsimd.partition_broadcast(rbkv[:],
                                                      rstdkv[0:1, sl])
                        state["rbkv"] = rbkv

                def group(cT, wb, wsh, dsts, rbkey, p):
                    def emit():
                        pp = pair.tile([128, 2 * SB], F32, name="pp",
                                       tag="pair")
                        nc.tensor.matmul(pp[:, 0:SB],
                                         wb[:, p * KC:(p + 1) * KC], cT[:, sl])
                        nc.tensor.matmul(pp[:, SB:2 * SB],
                                         wsh[:, p * KC:(p + 1) * KC], cT[:, sl])
                        t12 = t12p.tile([128, 2 * SB], F32, name="t12",
                                        tag="t12")
                        nc.vector.tensor_mul(t12[:], pp[:], cs_sl)
                        nc.vector.tensor_add(dsts[p][:, sl], t12[:, 0:SB],
                                             t12[:, SB:2 * SB])
                        rb = state.get(rbkey)
                        if rb is not None:
                            nc.vector.tensor_mul(dsts[p][:, sl],
                                                 dsts[p][:, sl], rb[:])
                    return emit

                def vtiles():
                    vps = pair.tile([128, 2 * SB], F32, name="vps", tag="pair")
                    for t in range(4):
                        g = sb * 4 + t
                        nc.tensor.matmul(vps[:, t * 256:(t + 1) * 256],
                                         ckvT[:, g * ST:(g + 1) * ST],
                                         uv_sb[:])
                    for t in range(4):
                        g = sb * 4 + t
                        vsrc = vps[:, t * 256:(t + 1) * 256].rearrange(
                            "p (h d) -> p h d", h=NH)
                        nc.vector.tensor_scalar_mul(
                            v_blocks[:, g, :, 0:HD], vsrc,
                            rstdk8[:, g:g + 1])

                def first():
                    bcast()
                    group(cqT, qb_sb, qsh_sb, (qT01, qT23), "rbq", 0)()
                chunks = [first,
                          group(ckvT, kb_sb, ksh_sb, (kT01, kT23), "rbkv", 0),
                          group(cqT, qb_sb, qsh_sb, (qT01, qT23), "rbq", 1),
                          group(ckvT, kb_sb, ksh_sb, (kT01, kT23), "rbkv", 1),
                          vtiles]
                return chunks

            def p4_units(j):
                units = []
                for t in range(4 * j, 4 * j + 4):
                    for dh in range(2):
                        def emit(t=t, dh=dh):
                            yp = pair.tile([128, 2 * SB], F32, name="yp",
                                           tag="pair")
                            for di in range(2):
                                dcol = dh * 1024 + di * SB
                                for c, oT in ((0, outT01), (1, outT23)):
                                    nc.tensor.matmul(
                                        yp[:, di * SB:(di + 1) * SB],
                                        oT[:, t * ST:(t + 1) * ST],
                                        wo_sb[:, c * D + dcol:c * D + dcol + SB],
                                        start=(c == 0), stop=(c == 1))
                            ysb = ysbp.tile([128, 2 * SB], F16, name="ysb",
                                            tag="ysb")
                            nc.vector.tensor_copy(ysb[:], yp[:])
                            nc.sync.dma_start(
                                y[t * ST:(t + 1) * ST,
                                  dh * 1024:(dh + 1) * 1024], ysb[:])
                        units.append(emit)
                return units

            def emit_scores(j, i):
                """Emit score matmuls + exp (+mask/tri) for key-tile i of
                block j, both head pairs. Returns the et tiles."""
                q0 = ST * (i - 4 * j) if (causal and i >= 4 * j) else 0
                ets = []
                mt = None
                if use_mask:
                    mt = mtp.tile([128, SB], F32, name="mt", tag="mt")
                    nc.sync.dma_start(
                        mt[:], maskT[i * ST:(i + 1) * ST,
                                     j * SB:(j + 1) * SB])
                for hp, (kTp, qTp) in enumerate(((kT01, qT01), (kT23, qT23))):
                    sc2 = pair.tile([128, 2 * SB], F32, name="sc2", tag="pair")
                    for half in range(2):
                        hs = slice(half * 64, half * 64 + 64)
                        nc.tensor.matmul(
                            sc2[:, half * SB + q0:(half + 1) * SB],
                            kTp[hs, i * ST:(i + 1) * ST],
                            qTp[hs, j * SB + q0:(j + 1) * SB])
                    if use_mask:
                        for half in range(2):
                            nc.vector.tensor_add(
                                sc2[:, half * SB:(half + 1) * SB],
                                sc2[:, half * SB:(half + 1) * SB], mt[:])
                    et = etp.tile([128, 2 * SB], BF16, name="et", tag="et")
                    scal = 0.125 if use_mask else rstdk8[:, i:i + 1]
                    if q0 == 0:
                        nc.scalar.activation(et[:], sc2[:], AF.Exp, scale=scal)
                    else:
                        for half in range(2):
                            rg = slice(half * SB + q0, (half + 1) * SB)
                            nc.scalar.activation(et[:, rg], sc2[:, rg],
                                                 AF.Exp, scale=scal)
                    if causal and i >= 4 * j:
                        for half in range(2):
                            rg = slice(half * SB + q0, half * SB + q0 + ST)
                            nc.vector.tensor_mul(et[:, rg], et[:, rg], tri_sb[:])
                    ets.append((et, q0))
                return ets

            def emit_pv(j, i, ets, first, last):
                for hp, (et, q0) in enumerate(ets):
                    for half in range(2):
                        h = 2 * hp + half
                        nc.tensor.matmul(
                            oacc[h][:, q0:SB],
                            v_sb[:, i * (NH * VW) + h * VW:
                                 i * (NH * VW) + (h + 1) * VW],
                            et[:, half * SB + q0:(half + 1) * SB],
                            start=first, stop=last)

            for c in p2_chunks(0):
                c()
            for j in range(NSB):
                ktiles = list(range(4 * j + 4)) if causal else list(range(NST))
                oacc = [oaccp.tile([VW, SB], F32, name=f"oa{h}", tag="oa")
                        for h in range(NH)]
                # fillers: next block's up-proj/rope first, then prev P4
                units = (p2_chunks(j + 1) if j + 1 < NSB else []) \
                    + (p4_units(j - 1) if j > 0 else [])
                ui = 0
                prev = None
                for wi, i in enumerate(ktiles):
                    ets = emit_scores(j, i)
                    # dispense fillers to keep PE fed; pace to finish in time
                    remaining_waves = len(ktiles) - wi
                    quota = -(-(len(units) - ui) // remaining_waves)
                    for _ in range(min(quota, 2)):
                        if ui < len(units):
                            units[ui]()
                            ui += 1
                    if prev is not None:
                        emit_pv(j, prev[0], prev[1],
                                prev[0] == ktiles[0], False)
                    prev = (i, ets)
                emit_pv(j, prev[0], prev[1], prev[0] == ktiles[0], True)
                while ui < len(units):
                    units[ui]()
                    ui += 1
                # block tail: softmax denominators + output scaling
                for h in range(NH):
                    den = rcp.tile([1, SB], F32, name="den", tag="rc")
                    nc.vector.tensor_copy(den[:], oacc[h][HD:VW, :])
                    rc = rcp.tile([1, SB], F32, name="rc", tag="rc")
                    nc.vector.reciprocal_approx_fast(out=rc[:], in_=den[:])
                    if dbg and h < 1:
                        o0 = j * SB
                        nc.vector.tensor_copy(
                            dendump[0:1, o0:o0 + SB], oacc[h][HD:VW, :])
                        nc.vector.tensor_copy(
                            rcdump[0:1, o0:o0 + SB], rc[:])
                    rbo = rbop.tile([HD, SB], F32, name="rbo", tag="rbo")
                    nc.gpsimd.partition_broadcast(rbo[:], rc[:])
                    dst = (outT01 if h < 2 else outT23)[
                        (h % 2) * HD:(h % 2 + 1) * HD,
                        j * SB:(j + 1) * SB]
                    nc.vector.tensor_mul(dst, oacc[h][0:HD, :], rbo[:])
            for u in p4_units(NSB - 1):
                u()
            if dbg:
                for nm, t in (("d_ckvT", ckvT), ("d_cqT", cqT),
                              ("d_kT01", kT01), ("d_kT23", kT23),
                              ("d_qT01", qT01), ("d_qT23", qT23),
                              ("d_rstdq", rstdq), ("d_rstdk8", rstdk8),
                              ("d_vsb", v_sb),
                              ("d_outT01", outT01), ("d_outT23", outT23),
                              ("d_den", dendump), ("d_rc", rcdump)):
                    nc.sync.dma_start(dbg[nm], t[:])

    nc.finalize()
    return nc


_NC_CACHE = {}


def _get_nc(causal, use_mask):
    key = (causal, use_mask)
    if key not in _NC_CACHE:
        _NC_CACHE[key] = _build_nc(causal, use_mask)
    return _NC_CACHE[key]


def _prep_inputs(x, cos, sin, mask, w_kv_down, kv_norm_w, w_uk, w_ur, w_uv,
                 w_q_down, q_norm_w, w_uq, w_qr, w_o, use_mask):
    """Build the 8 per-core input maps (host-side shard + fold)."""
    import ml_dtypes as md
    f = np.float32
    x = np.asarray(x, f)
    cos = np.asarray(cos, f)
    sin = np.asarray(sin, f)
    w_kv_down = np.asarray(w_kv_down, f)
    w_q_down = np.asarray(w_q_down, f)
    kv_norm_w = np.asarray(kv_norm_w, f)
    q_norm_w = np.asarray(q_norm_w, f)
    w_uk_e = np.asarray(w_uk, f) * kv_norm_w[:, None]
    w_ur_e = np.asarray(w_ur, f) * kv_norm_w[:, None]
    w_uv_e = np.asarray(w_uv, f) * kv_norm_w[:, None]
    w_uq_e = np.asarray(w_uq, f) * q_norm_w[:, None]
    w_qr_e = np.asarray(w_qr, f) * q_norm_w[:, None]
    w_o = np.asarray(w_o, f)

    wkv = np.ascontiguousarray(
        w_kv_down.reshape(NKC, KC, R).transpose(1, 0, 2).reshape(KC, D))
    wq = np.ascontiguousarray(
        w_q_down.reshape(NKC, KC, R).transpose(1, 0, 2).reshape(KC, D))
    cosT = np.ascontiguousarray(cos.T)                 # [32, S]
    sinT = np.ascontiguousarray(sin.T)
    sinSg = np.concatenate([-sinT[:DR // 2], sinT[DR // 2:]], axis=0)
    one32 = np.ones((DR, S), np.float32)
    zero32 = np.zeros((DR, S), np.float32)
    # pair-tensor rope tables: nope rows pass through (cos=1, sin=0)
    cosPt = np.concatenate([one32, cosT, one32, cosT], axis=0)   # [128, S]
    sinPt = np.concatenate([zero32, sinSg, zero32, sinSg], axis=0)
    # s-block interleave: [cos_blk0 | sin_blk0 | cos_blk1 | sin_blk1 | ...]
    cssin = np.empty((128, 2 * S), np.float32)
    for sb in range(NSB):
        cssin[:, sb * 2 * SB:sb * 2 * SB + SB] = \
            cosPt[:, sb * SB:(sb + 1) * SB]
        cssin[:, sb * 2 * SB + SB:(sb + 1) * 2 * SB] = \
            sinPt[:, sb * SB:(sb + 1) * SB]
    cssin = np.ascontiguousarray(cssin)
    # rope shift permutation within each head's 32 cols
    perm = np.concatenate([np.arange(16, 32), np.arange(0, 16)])

    # x: [b] -> transpose -> s-block-major [NSB, D, SB]
    xT4b = []
    for b in range(B):
        xT = x[b].T                                      # [D, S]
        xT4 = np.ascontiguousarray(
            xT.reshape(D, NSB, SB).transpose(1, 0, 2)).astype(md.bfloat16)
        xT4b.append(xT4)
    maskT8 = None
    if use_mask:
        m = np.asarray(mask, f).reshape(S, S)
        maskT8 = np.ascontiguousarray(m.T) * 8.0

    in_maps = []
    z32 = np.zeros((R, DN), np.float32)
    for core in range(NCORES):
        b, g = core // 4, core % 4
        cs = slice(g * NH * DN, (g + 1) * NH * DN)      # 128-wide col slice
        vs = slice(g * NH * HD, (g + 1) * NH * HD)      # 256-wide
        uk_l = w_uk_e[:, cs].reshape(R, NH, DN)
        ur_l = w_ur_e[:, cs].reshape(R, NH, DR)
        urs_l = ur_l[:, :, perm]
        uq_l = w_uq_e[:, cs].reshape(R, NH, DN)
        qr_l = w_qr_e[:, cs].reshape(R, NH, DR)
        qrs_l = qr_l[:, :, perm]

        def pair(nope, rope):
            cols = []
            for h in range(NH):
                cols += [nope[:, h], rope[:, h]]
            return np.ascontiguousarray(np.concatenate(cols, axis=1))

        def pair_sh(sh):
            cols = []
            for h in range(NH):
                cols += [z32, sh[:, h]]
            return np.ascontiguousarray(np.concatenate(cols, axis=1))

        wo_loc = w_o[g * NH * HD:(g + 1) * NH * HD]     # [256, D]
        wo_r = np.ascontiguousarray(
            wo_loc.reshape(2, KC, D).transpose(1, 0, 2).reshape(KC, 2 * D)
        ).astype(md.bfloat16)
        m_ = {
            "xT4": xT4b[b],
            "wkv": wkv.astype(md.bfloat16), "wq": wq.astype(md.bfloat16),
            "kb": pair(uk_l, ur_l).astype(md.bfloat16),
            "ksh": pair_sh(urs_l).astype(md.bfloat16),
            "qb": pair(uq_l, qr_l).astype(md.bfloat16),
            "qsh": pair_sh(qrs_l).astype(md.bfloat16),
            "uv": np.ascontiguousarray(w_uv_e[:, vs]).astype(md.bfloat16),
            "wo": wo_r,
            "cssin": cssin,
        }
        if use_mask:
            m_["maskT"] = maskT8
        in_maps.append(m_)
    return in_maps


def _classify_mask(mask):
    m = np.asarray(mask, np.float32).reshape(S, S)
    if not np.any(m):
        return False, False          # dense, no mask
    causal_ref = np.where(
        np.tril(np.ones((S, S), dtype=bool)), np.float32(0.0),
        np.float32(-1e9))
    if np.array_equal(m, causal_ref):
        return True, False           # structural causal
    return False, True               # generic additive mask


LAST_RESULTS = None


def kernel(**inputs):
    global LAST_RESULTS
    from concourse.bass_utils import run_bass_kernel_spmd
    causal, use_mask = _classify_mask(inputs["mask"])
    nc = _get_nc(causal, use_mask)
    in_maps = _prep_inputs(
        inputs["x"], inputs["cos"], inputs["sin"], inputs["mask"],
        inputs["w_kv_down"], inputs["kv_norm_w"], inputs["w_uk"],
        inputs["w_ur"], inputs["w_uv"], inputs["w_q_down"],
        inputs["q_norm_w"], inputs["w_uq"], inputs["w_qr"], inputs["w_o"],
        use_mask)
    res = run_bass_kernel_spmd(nc, in_maps, list(range(NCORES)))
    LAST_RESULTS = res
    parts = [np.asarray(res.results[c]["y"], np.float32)
             for c in range(NCORES)]
    out = np.empty((B, S, D), np.float32)
    for b in range(B):
        out[b] = parts[4 * b] + parts[4 * b + 1] + parts[4 * b + 2] \
            + parts[4 * b + 3]
    return out
